# revision 2
# baseline (speedup 1.0000x reference)
"""Self-contained Trainium2 Bass kernel for nn_DecoderLayerWithMoE.

kernel(**inputs) takes the FULL unsharded inputs (as produced by
setup_inputs()) and returns (x, aux) matching reference(**inputs).
Work is distributed across 8 NeuronCores: data-parallel attention,
expert-parallel MoE with AllGather / ReduceScatter / AllReduce
collectives. See build_kernel docstring for the layout details.
"""

import sys
sys.path.insert(0, "/opt/trn_rl_repo")

"""Builder for the DecoderLayerWithMoE distributed Bass kernel.

Distribution over 8 cores:
  - Attention + LayerNorms: data-parallel. Core c handles batch b=c//(NC//B),
    query-token chunk (c % (NC//B)) of size CHUNK = B*S/NC. K/V computed
    redundantly from the full per-batch sequence (inputs replicated per batch).
  - MoE: expert-parallel dense. Core c owns expert e=c, computes its expert for
    ALL tokens, scales by the (zero-for-unselected) combine gate, and a
    ReduceScatter(add) returns each core its own token chunk.
  - Router runs in fp32 from exact (pre-AllGather) logits so top-2 decisions
    match the reference up to fp32 rounding.
  - Aux loss: per-core partial sums over its chunk + tiny AllReduce.

Layout conventions:
  fm = feature-major [D on partitions, tokens on free]  (matmul-friendly)
  tm = token-major  [tokens on partitions, D on free]   (LayerNorm-friendly)
"""

import numpy as np
import ml_dtypes

import concourse.bass as bass
import concourse.mybir as mybir
import concourse.tile as tile
from concourse.masks import make_identity
from concourse.tile_rust import add_dep_helper

F32 = mybir.dt.float32
BF16 = mybir.dt.bfloat16
AF = mybir.ActivationFunctionType
ALU = mybir.AluOpType
AX = mybir.AxisListType
P = 128


def part_bcast(ap, p):
    """Broadcast a 1-D dram AP along a new leading partition dim."""
    return bass.AP(tensor=ap.tensor, offset=ap.offset,
                   ap=[[0, p]] + [list(x) for x in ap.ap])


class Cfg:
    def __init__(self, D=1024, H=16, F=4096, E=8, K=2, B=2, S=2048, NC=8):
        self.D, self.H, self.F, self.E, self.K, self.B, self.S, self.NC = (
            D, H, F, E, K, B, S, NC)
        self.DH = D // H
        self.NTOK = B * S
        self.CHUNK = self.NTOK // NC     # tokens per core
        self.KV = S                      # kv length per batch
        self.D_T = D // P
        self.F_T = F // P
        self.CH_T = self.CHUNK // P
        self.KV_T = self.KV // P
        self.KV_N = max(1, self.KV // 512)   # 512-wide free slices over KV
        self.D_N = max(1, D // 512)          # 512-wide free slices over D
        self.HPG = 512 // self.DH            # heads per group
        self.NG = self.H // self.HPG         # head groups
        assert D % P == 0 and F % P == 0 and self.CHUNK % P == 0
        assert self.DH == 64 and E == 8 and K == 2
        assert self.CHUNK <= 512 and self.KV % 512 == 0 and D % 512 == 0
        assert self.H * self.DH == D


def build_kernel(tc, outs, ins, cfg):
    nc = tc.nc
    c = cfg

    with tc.tile_pool(name="const", bufs=1) as const, \
         tc.tile_pool(name="x2res", bufs=c.CH_T) as x2res_pool, \
         tc.tile_pool(name="dram", bufs=1, space="DRAM") as dram:

        # ---------------- constants ----------------
        ident_f = const.tile([P, P], F32, name="ident_f")
        make_identity(nc, ident_f)
        ident_b = const.tile([P, P], BF16, name="ident_b")
        make_identity(nc, ident_b)
        ones_row = const.tile([1, P], F32, name="ones_row")
        nc.vector.memset(ones_row, 1.0)
        ones_col = const.tile([P, 1], F32, name="ones_col")
        nc.vector.memset(ones_col, 1.0)
        rb_bc = const.tile([P, c.E], F32, name="rb_bc")
        nc.gpsimd.dma_start(out=rb_bc, in_=part_bcast(ins["rb"], P))
        sel_bc = const.tile([P, c.E], F32, name="sel_bc")
        nc.gpsimd.dma_start(out=sel_bc, in_=part_bcast(ins["sel"], P))
        rw_sb = []
        for k in range(c.D_T):
            t = const.tile([P, c.E], F32, name=f"rw_sb{k}")
            nc.sync.dma_start(out=t, in_=ins["rw"][k * P:(k + 1) * P, :])
            rw_sb.append(t)

        # ---------------- collective DRAM buffers ----------------
        cc_ag_in = dram.tile([c.D, c.CHUNK], BF16, name="cc_ag_in")
        cc_ag_out = dram.tile([c.NC * c.D, c.CHUNK], BF16, name="cc_ag_out",
                              addr_space="Shared")
        cc_lg_in = dram.tile([c.CHUNK, c.E], F32, name="cc_lg_in")
        cc_lg_out = dram.tile([c.NC * c.CHUNK, c.E], F32, name="cc_lg_out",
                              addr_space="Shared")
        cc_y_in = dram.tile([c.NTOK, c.D], F32, name="cc_y_in")
        cc_y_out = dram.tile([c.CHUNK, c.D], F32, name="cc_y_out")
        cc_aux_in = dram.tile([c.E, 2], F32, name="cc_aux_in")
        cc_aux_out = dram.tile([c.E, 2], F32, name="cc_aux_out",
                               addr_space="Shared")

        # ---------------- shared helpers ----------------
        def bcast_row(pl, name, width, tag="lnp", bufs=3):
            t = pl.tile([P, width], F32, name=f"br_{name}", tag=tag, bufs=bufs)
            nc.gpsimd.dma_start(out=t, in_=part_bcast(ins[name], P))
            return t

        def layernorm(src, dst, g_bc, b_bc, sqpool, stat):
            """token-major LN over free dim D (in-place safe: dst != src)."""
            sum_x = stat.tile([P, 1], F32, name="ln_sumx", tag="st")
            nc.vector.reduce_sum(sum_x, src, axis=AX.X)
            sqd = sqpool.tile([P, c.D], BF16, name="ln_sqd", tag="sq", bufs=2)
            sum_sq = stat.tile([P, 1], F32, name="ln_sumsq", tag="st")
            nc.vector.scalar_tensor_tensor(
                out=sqd, in0=src, scalar=1.0, in1=src,
                op0=ALU.mult, op1=ALU.mult, accum_out=sum_sq)
            mean = stat.tile([P, 1], F32, name="ln_mean", tag="st")
            nc.vector.tensor_scalar(out=mean, in0=sum_x, scalar1=1.0 / c.D,
                                    scalar2=None, op0=ALU.mult)
            msq = stat.tile([P, 1], F32, name="ln_msq", tag="st")
            nc.vector.tensor_tensor(out=msq, in0=mean, in1=mean, op=ALU.mult)
            # var + eps = sum_sq/D - mean^2 + eps
            var = stat.tile([P, 1], F32, name="ln_var", tag="st")
            nc.vector.tensor_scalar(out=var, in0=sum_sq, scalar1=1.0 / c.D,
                                    scalar2=1e-5, op0=ALU.mult, op1=ALU.add)
            vareps = stat.tile([P, 1], F32, name="ln_vareps", tag="st")
            nc.vector.tensor_tensor(out=vareps, in0=var, in1=msq,
                                    op=ALU.subtract)
            std = stat.tile([P, 1], F32, name="ln_std", tag="st")
            nc.scalar.sqrt(std, vareps)
            rstd = stat.tile([P, 1], F32, name="ln_rstd", tag="st")
            nc.vector.reciprocal(rstd, std)
            nc.vector.tensor_scalar(out=dst, in0=src, scalar1=mean,
                                    scalar2=rstd, op0=ALU.subtract,
                                    op1=ALU.mult)
            nc.vector.tensor_tensor(out=dst, in0=dst, in1=g_bc, op=ALU.mult)
            nc.vector.tensor_tensor(out=dst, in0=dst, in1=b_bc, op=ALU.add)

        def router_probs(lg_ap, scr, stat, out_ind=None, out_cmb=None):
            """softmax over E + top-2 machinery from exact f32 logits."""
            m1a = stat.tile([P, 1], F32, name="rt_m1a", tag="st")
            nc.vector.reduce_max(m1a, lg_ap, axis=AX.X)
            negm = stat.tile([P, 1], F32, name="rt_negm", tag="st")
            nc.vector.tensor_scalar(out=negm, in0=m1a, scalar1=-1.0,
                                    scalar2=None, op0=ALU.mult)
            et = scr.tile([P, c.E], F32, name="rt_exp", tag="rtE", bufs=6)
            den = stat.tile([P, 1], F32, name="rt_den", tag="st")
            nc.scalar.activation(et, lg_ap, AF.Exp, bias=negm, accum_out=den)
            rd = stat.tile([P, 1], F32, name="rt_rd", tag="st")
            nc.vector.reciprocal(rd, den)
            probs = scr.tile([P, c.E], F32, name="rt_probs", tag="rtE", bufs=6)
            nc.vector.tensor_scalar(out=probs, in0=et, scalar1=rd,
                                    scalar2=None, op0=ALU.mult)
            m1 = stat.tile([P, 1], F32, name="rt_m1", tag="st")
            nc.vector.reduce_max(m1, probs, axis=AX.X)
            lt = scr.tile([P, c.E], F32, name="rt_lt", tag="rtE", bufs=6)
            nc.vector.tensor_scalar(out=lt, in0=probs, scalar1=m1,
                                    scalar2=None, op0=ALU.is_lt)
            pm = scr.tile([P, c.E], F32, name="rt_pm", tag="rtE", bufs=6)
            nc.vector.tensor_tensor(out=pm, in0=probs, in1=lt, op=ALU.mult)
            m2 = stat.tile([P, 1], F32, name="rt_m2", tag="st")
            nc.vector.reduce_max(m2, pm, axis=AX.X)
            if out_ind is not None:
                nc.vector.tensor_scalar(out=out_ind, in0=probs, scalar1=m2,
                                        scalar2=None, op0=ALU.is_ge)
            if out_cmb is not None:
                s12 = stat.tile([P, 1], F32, name="rt_s12", tag="st")
                nc.vector.tensor_tensor(out=s12, in0=m1, in1=m2, op=ALU.add)
                gsc = stat.tile([P, 1], F32, name="rt_gsc", tag="st")
                nc.vector.reciprocal(gsc, s12)
                pse = scr.tile([P, c.E], F32, name="rt_pse", tag="rtE", bufs=6)
                nc.vector.tensor_tensor(out=pse, in0=probs, in1=sel_bc,
                                        op=ALU.mult)
                pe = stat.tile([P, 1], F32, name="rt_pe", tag="st")
                nc.vector.reduce_sum(pe, pse, axis=AX.X)
                ind = stat.tile([P, 1], F32, name="rt_ind", tag="st")
                nc.vector.tensor_scalar(out=ind, in0=pe, scalar1=m2,
                                        scalar2=None, op0=ALU.is_ge)
                t1 = stat.tile([P, 1], F32, name="rt_t1", tag="st")
                nc.vector.tensor_tensor(out=t1, in0=pe, in1=gsc, op=ALU.mult)
                nc.vector.tensor_tensor(out=out_cmb, in0=t1, in1=ind,
                                        op=ALU.mult)
            return probs

        # ================= phases A + B: attention =================
        with tc.tile_pool(name="ab_big", bufs=1) as abp, \
             tc.tile_pool(name="ab_stat", bufs=24) as stat, \
             tc.tile_pool(name="ab_ps", bufs=4, space="PSUM") as ps_big, \
             tc.tile_pool(name="ab_psb", bufs=2, space="PSUM") as ps_small:

            def T(shape, dt, name, tag, bufs):
                return abp.tile(shape, dt, name=name, tag=tag, bufs=bufs)

            def bias_cols(name):
                t = abp.tile([P, c.D_T], F32, name=f"bc_{name}", tag="bcol",
                             bufs=8)
                nc.sync.dma_start(out=t,
                                  in_=ins[name].rearrange("(t p) -> p t", p=P))
                return t

            def attention(pfx, q_src, kv_dram, resid_in, ln_g, ln_b,
                          want_logits):
                tc.no_sync_barrier()
                EH = c.DH + 1
                bq, bk, bo = (bias_cols(pfx + "_bq"), bias_cols(pfx + "_bk"),
                              bias_cols(pfx + "_bo"))
                bv_bc = bcast_row(abp, pfx + "_bv", c.D)
                g_bc = bcast_row(abp, ln_g, c.D)
                b_bc = bcast_row(abp, ln_b, c.D)

                xt = []
                for k in range(c.D_T):
                    t = T([P, c.KV], BF16, f"{pfx}_xt{k}", "xt", c.D_T)
                    nc.sync.dma_start(out=t, in_=kv_dram[k * P:(k + 1) * P, :])
                    xt.append(t)

                def load_w_full(wname):
                    w = []
                    for k in range(c.D_T):
                        t = T([P, c.D], BF16, f"{wname}{k}", "wbig", c.D_T)
                        nc.sync.dma_start(out=t,
                                          in_=ins[wname][k * P:(k + 1) * P, :])
                        w.append(t)
                    return w

                # Q projection (fm)
                wq = load_w_full(pfx + "_wq")
                q_sb = []
                for m in range(c.D_T):
                    ps = ps_big.tile([P, c.CHUNK], F32, name="ps_q", tag="ps")
                    for k in range(c.D_T):
                        nc.tensor.matmul(ps, wq[k][:, m * P:(m + 1) * P],
                                         q_src[k], start=(k == 0),
                                         stop=(k == c.D_T - 1))
                    t = T([P, c.CHUNK], BF16, f"{pfx}_q{m}", "q", c.D_T)
                    nc.scalar.activation(t, ps, AF.Identity,
                                         bias=bq[:, m:m + 1])
                    q_sb.append(t)

                o_sb = [T([P, c.CHUNK], BF16, f"{pfx}_o{m}", "o", c.D_T)
                        for m in range(c.D_T)]

                for g in range(c.NG):
                    if g:
                        tc.no_sync_barrier()
                    gsl = slice(g * 512, (g + 1) * 512)
                    # K for this head group (fm rows g*512 .. +512)
                    wk_g = []
                    for k in range(c.D_T):
                        t = T([P, 512], BF16, f"{pfx}_wk{g}_{k}", "wsm",
                              c.D_T + 1)
                        nc.sync.dma_start(out=t,
                                          in_=ins[pfx + "_wk"][k * P:(k + 1) * P,
                                                               gsl])
                        wk_g.append(t)
                    k_g = []
                    for mi in range(512 // P):
                        t = T([P, c.KV], BF16, f"{pfx}_k{g}_{mi}", "k", 5)
                        for ns in range(c.KV_N):
                            nsl = slice(ns * 512, (ns + 1) * 512)
                            ps = ps_big.tile([P, 512], F32, name="ps_k",
                                             tag="ps")
                            for k in range(c.D_T):
                                nc.tensor.matmul(ps, wk_g[k][:, mi * P:(mi + 1) * P],
                                                 xt[k][:, nsl], start=(k == 0),
                                                 stop=(k == c.D_T - 1))
                            nc.scalar.activation(
                                t[:, nsl], ps, AF.Identity,
                                bias=bk[:, (g * 512 // P) + mi:
                                        (g * 512 // P) + mi + 1])
                        k_g.append(t)
                    # V for this head group (tm, interleaved ones col)
                    wv_g = []
                    for k in range(c.D_T):
                        t = T([P, 512], BF16, f"{pfx}_wv{g}_{k}", "wsm",
                              c.D_T + 1)
                        nc.sync.dma_start(out=t,
                                          in_=ins[pfx + "_wv"][k * P:(k + 1) * P,
                                                               gsl])
                        wv_g.append(t)
                    v_g = []
                    for tt in range(c.KV_T):
                        t = T([P, c.HPG * EH], BF16, f"{pfx}_v{g}_{tt}", "v",
                              c.KV_T + 1)
                        v3 = t.rearrange("p (h e) -> p h e", e=EH)
                        nc.vector.memset(v3[:, :, c.DH:EH], 1.0)
                        ps = ps_big.tile([P, 512], F32, name="ps_v", tag="ps")
                        for k in range(c.D_T):
                            nc.tensor.matmul(ps, xt[k][:, tt * P:(tt + 1) * P],
                                             wv_g[k], start=(k == 0),
                                             stop=(k == c.D_T - 1))
                        view = v3[:, :, 0:c.DH]
                        nc.scalar.activation(view, ps, AF.Identity)
                        bvv = bv_bc[:, gsl].rearrange("p (h d) -> p h d",
                                                      d=c.DH)
                        nc.vector.tensor_tensor(out=view, in0=view, in1=bvv,
                                                op=ALU.add)
                        v_g.append(t)
                    # heads in this group
                    for hl in range(c.HPG):
                        h = g * c.HPG + hl
                        ki, ro = (hl * c.DH) // P, (hl * c.DH) % P
                        q_i, q_ro = (h * c.DH) // P, (h * c.DH) % P
                        q_ap = q_sb[q_i][q_ro:q_ro + c.DH, :]
                        ps_o = ps_big.tile([c.DH + 1, c.CHUNK], F32,
                                           name="ps_o", tag="ps")
                        av_insts = []
                        for kt in range(c.KV_T):
                            ps_s = ps_big.tile([P, c.CHUNK], F32, name="ps_s",
                                               tag="ps")
                            sc = nc.tensor.matmul(
                                ps_s, k_g[ki][ro:ro + c.DH, kt * P:(kt + 1) * P],
                                q_ap, start=True, stop=True)
                            if kt >= 2:
                                # keep scores <=2 ahead of AV so pt/ps slots
                                # can't wedge the pipeline
                                add_dep_helper(sc.ins, av_insts[kt - 2].ins,
                                               sync=False, reason="pt pacing")
                            ptt = T([P, c.CHUNK], BF16, "pt", "pt", 4)
                            nc.scalar.activation(ptt, ps_s, AF.Exp,
                                                 scale=float(1.0 / np.sqrt(c.DH)))
                            av = nc.tensor.matmul(
                                ps_o, v_g[kt][:, hl * EH:(hl + 1) * EH],
                                ptt, start=(kt == 0), stop=(kt == c.KV_T - 1))
                            av_insts.append(av)
                        recip = stat.tile([1, c.CHUNK], F32, name="att_recip",
                                          tag="arec", bufs=2)
                        nc.vector.reciprocal(recip, ps_o[c.DH:c.DH + 1, :])
                        ps_b = ps_small.tile([c.DH, c.CHUNK], F32, name="ps_b",
                                             tag="psb")
                        nc.tensor.matmul(ps_b, ones_row[0:1, 0:c.DH], recip,
                                         start=True, stop=True)
                        rbc = T([c.DH, c.CHUNK], BF16, "rbc", "abc", 3)
                        nc.scalar.copy(rbc, ps_b)
                        oi, oro = (h * c.DH) // P, (h * c.DH) % P
                        nc.vector.tensor_tensor(
                            out=o_sb[oi][oro:oro + c.DH, :],
                            in0=ps_o[0:c.DH, :], in1=rbc, op=ALU.mult)

                # out projection (fm, bf16) + transpose + residual + LN (tm)
                wo = load_w_full(pfx + "_wo")
                ao = []
                for m in range(c.D_T):
                    ps = ps_big.tile([P, c.CHUNK], F32, name="ps_ao", tag="ps")
                    for k in range(c.D_T):
                        nc.tensor.matmul(ps, wo[k][:, m * P:(m + 1) * P],
                                         o_sb[k], start=(k == 0),
                                         stop=(k == c.D_T - 1))
                    t = T([P, c.CHUNK], BF16, f"{pfx}_ao{m}", "ao", c.D_T)
                    nc.scalar.activation(t, ps, AF.Identity,
                                         bias=bo[:, m:m + 1])
                    ao.append(t)

                x_next = []
                for j in range(c.CH_T):
                    xpre = T([P, c.D], F32, f"{pfx}_xpre{j}", "resid", 5)
                    for m in range(c.D_T):
                        ps_t = ps_small.tile([P, P], BF16, name="ps_t",
                                             tag="psb")
                        nc.tensor.transpose(ps_t, ao[m][:, j * P:(j + 1) * P],
                                            ident_b)
                        nc.vector.tensor_tensor(
                            out=xpre[:, m * P:(m + 1) * P], in0=ps_t,
                            in1=resid_in[j][:, m * P:(m + 1) * P], op=ALU.add)
                    if want_logits:
                        dst = x2res_pool.tile([P, c.D], F32, name=f"x2tm{j}",
                                              tag="x2")
                    else:
                        dst = T([P, c.D], F32, f"{pfx}_xn{j}", "resid", 5)
                    layernorm(xpre, dst, g_bc, b_bc, abp, stat)
                    x_next.append(dst)

                # transpose x_next -> fm bf16 (+ f32 logits if requested)
                x_fm_bf = [T([P, c.CHUNK], BF16, f"{pfx}_xfmb{m}", "xfm",
                             c.D_T) for m in range(c.D_T)]
                for j in range(c.CH_T):
                    tmp32 = []
                    for m in range(c.D_T):
                        ps_t = ps_small.tile([P, P], F32, name="ps_t2",
                                             tag="psb")
                        nc.tensor.transpose(ps_t,
                                            x_next[j][:, m * P:(m + 1) * P],
                                            ident_f)
                        nc.scalar.copy(x_fm_bf[m][:, j * P:(j + 1) * P], ps_t)
                        if want_logits:
                            tf = T([P, P], F32, f"t32_{m}", "t32", c.D_T + 1)
                            nc.vector.tensor_copy(tf, ps_t)
                            tmp32.append(tf)
                    if want_logits:
                        ps_lg = ps_small.tile([P, c.E], F32, name="ps_lg",
                                              tag="psb")
                        for k in range(c.D_T):
                            nc.tensor.matmul(ps_lg, tmp32[k], rw_sb[k],
                                             start=(k == 0),
                                             stop=(k == c.D_T - 1))
                        lg = abp.tile([P, c.E], F32, name="lg_sb", tag="rtE",
                                      bufs=6)
                        nc.vector.tensor_tensor(out=lg, in0=ps_lg, in1=rb_bc,
                                                op=ALU.add)
                        nc.sync.dma_start(out=cc_lg_in[j * P:(j + 1) * P, :],
                                          in_=lg)
                return x_next, x_fm_bf

            # ---- run the two attention phases ----
            xq = []
            for k in range(c.D_T):
                t = abp.tile([P, c.CHUNK], BF16, name=f"xq{k}", tag="xq",
                             bufs=c.D_T)
                nc.sync.dma_start(out=t, in_=ins["xq"][k * P:(k + 1) * P, :])
                xq.append(t)
            x_tm = []
            for j in range(c.CH_T):
                t = abp.tile([P, c.D], F32, name=f"xtm{j}", tag="resid",
                             bufs=5)
                nc.sync.dma_start(out=t, in_=ins["x_tm"][j * P:(j + 1) * P, :])
                x_tm.append(t)

            x1_tm, x1_fm = attention("sa", xq, ins["xkv"], x_tm,
                                     "n1_g", "n1_b", False)
            x2_tm, x2_fm = attention("ca", x1_fm, ins["enc"], x1_tm,
                                     "n2_g", "n2_b", True)

            for m in range(c.D_T):
                nc.sync.dma_start(out=cc_ag_in[m * P:(m + 1) * P, :],
                                  in_=x2_fm[m])

        nc.gpsimd.collective_compute(
            "AllGather", ALU.bypass, replica_groups=[list(range(c.NC))],
            ins=[cc_ag_in[:]], outs=[cc_ag_out[:]])
        nc.gpsimd.collective_compute(
            "AllGather", ALU.bypass, replica_groups=[list(range(c.NC))],
            ins=[cc_lg_in[:]], outs=[cc_lg_out[:]])

        # ================= phase C: MoE (dense expert-parallel) =============
        with tc.tile_pool(name="moe", bufs=1) as mp, \
             tc.tile_pool(name="moe_stat", bufs=24) as mstat, \
             tc.tile_pool(name="moe_ps", bufs=4, space="PSUM") as mps, \
             tc.tile_pool(name="moe_psb", bufs=2, space="PSUM") as mpsb:

            w1_sb = []
            for k in range(c.D_T):
                t = mp.tile([P, c.F], BF16, name=f"w1sb{k}", tag="w1",
                            bufs=c.D_T)
                nc.sync.dma_start(out=t, in_=ins["w1"][k * P:(k + 1) * P, :])
                w1_sb.append(t)
            w2_sb = []
            for f in range(c.F_T):
                t = mp.tile([P, c.D], BF16, name=f"w2sb{f}", tag="w2",
                            bufs=c.F_T)
                nc.sync.dma_start(out=t, in_=ins["w2"][f * P:(f + 1) * P, :])
                w2_sb.append(t)
            b1_sb = mp.tile([P, c.F_T], F32, name="b1_sb", tag="b1", bufs=1)
            nc.sync.dma_start(out=b1_sb,
                              in_=ins["b1"].rearrange("(t p) -> p t", p=P))
            b2_bc = bcast_row(mp, "b2", c.D, tag="lnp", bufs=1)

            for blk in range(c.NC):
                if blk:
                    tc.no_sync_barrier()
                xb = []
                for k in range(c.D_T):
                    t = mp.tile([P, c.CHUNK], BF16, name=f"xb{k}", tag="xb",
                                bufs=c.D_T + 2)
                    nc.sync.dma_start(
                        out=t, in_=cc_ag_out[blk * c.D + k * P:
                                             blk * c.D + (k + 1) * P, :])
                    xb.append(t)
                cmb = []
                for tt in range(c.CH_T):
                    lg = mp.tile([P, c.E], F32, name="moe_lg", tag="rtE",
                                 bufs=6)
                    row = blk * c.CHUNK + tt * P
                    nc.sync.dma_start(out=lg, in_=cc_lg_out[row:row + P, :])
                    cm = mp.tile([P, 1], F32, name=f"cmb{tt}", tag="cmb",
                                 bufs=c.CH_T + 2)
                    router_probs(lg, mp, mstat, out_cmb=cm)
                    cmb.append(cm)
                h_bf = []
                for f in range(c.F_T):
                    ps = mps.tile([P, c.CHUNK], F32, name="ps_h", tag="ps")
                    for k in range(c.D_T):
                        nc.tensor.matmul(ps, w1_sb[k][:, f * P:(f + 1) * P],
                                         xb[k], start=(k == 0),
                                         stop=(k == c.D_T - 1))
                    t = mp.tile([P, c.CHUNK], BF16, name=f"h{f}", tag="h",
                                bufs=c.F_T + 2)
                    nc.scalar.activation(t, ps, AF.Relu,
                                         bias=b1_sb[:, f:f + 1])
                    h_bf.append(t)
                for tt in range(c.CH_T):
                    for dn in range(c.D_N):
                        ps = mps.tile([P, 512], F32, name="ps_y", tag="ps")
                        for f in range(c.F_T):
                            nc.tensor.matmul(
                                ps, h_bf[f][:, tt * P:(tt + 1) * P],
                                w2_sb[f][:, dn * 512:(dn + 1) * 512],
                                start=(f == 0), stop=(f == c.F_T - 1))
                        yt = mp.tile([P, 512], F32, name="yt", tag="y", bufs=4)
                        nc.vector.tensor_tensor(
                            out=yt, in0=ps,
                            in1=b2_bc[:, dn * 512:(dn + 1) * 512], op=ALU.add)
                        yt2 = mp.tile([P, 512], F32, name="yt2", tag="y",
                                      bufs=4)
                        nc.vector.tensor_scalar(out=yt2, in0=yt,
                                                scalar1=cmb[tt], scalar2=None,
                                                op0=ALU.mult)
                        row = blk * c.CHUNK + tt * P
                        nc.sync.dma_start(
                            out=cc_y_in[row:row + P, dn * 512:(dn + 1) * 512],
                            in_=yt2)

            nc.gpsimd.collective_compute(
                "ReduceScatter", ALU.add, replica_groups=[list(range(c.NC))],
                ins=[cc_y_in[:]], outs=[cc_y_out[:]])

        # ================= tail: LN3 + aux loss =================
        with tc.tile_pool(name="tl", bufs=1) as tp, \
             tc.tile_pool(name="tl_stat", bufs=24) as tstat, \
             tc.tile_pool(name="tl_ps", bufs=2, space="PSUM") as tps, \
             tc.tile_pool(name="tl_psb", bufs=2, space="PSUM") as tpsb:

            g3_bc = bcast_row(tp, "n3_g", c.D, tag="lnp", bufs=2)
            b3_bc = bcast_row(tp, "n3_b", c.D, tag="lnp", bufs=2)

            x3_tm = []
            for j in range(c.CH_T):
                yj = tp.tile([P, c.D], F32, name=f"yj{j}", tag="resid", bufs=4)
                nc.sync.dma_start(out=yj, in_=cc_y_out[j * P:(j + 1) * P, :])
                x3pre = tp.tile([P, c.D], F32, name=f"x3pre{j}", tag="resid",
                                bufs=4)
                nc.vector.tensor_tensor(out=x3pre, in0=yj, in1=x2_tm[j],
                                        op=ALU.add)
                x3 = tp.tile([P, c.D], F32, name=f"x3_{j}", tag="x3",
                             bufs=c.CH_T)
                layernorm(x3pre, x3, g3_bc, b3_bc, tp, tstat)
                nc.sync.dma_start(out=outs["out_x"][j * P:(j + 1) * P, :],
                                  in_=x3)
                x3_tm.append(x3)

            ps_imp = tps.tile([c.E, 1], F32, name="ps_imp", tag="acc")
            ps_load = tps.tile([c.E, 1], F32, name="ps_load", tag="acc")
            for j in range(c.CH_T):
                tmp32 = []
                for m in range(c.D_T):
                    ps_t = tpsb.tile([P, P], F32, name="ps_t3", tag="psb")
                    nc.tensor.transpose(ps_t, x3_tm[j][:, m * P:(m + 1) * P],
                                        ident_f)
                    tf = tp.tile([P, P], F32, name=f"t33_{m}", tag="t32",
                                 bufs=c.D_T + 2)
                    nc.vector.tensor_copy(tf, ps_t)
                    tmp32.append(tf)
                ps_lg = tpsb.tile([P, c.E], F32, name="ps_lg3", tag="psb")
                for k in range(c.D_T):
                    nc.tensor.matmul(ps_lg, tmp32[k], rw_sb[k],
                                     start=(k == 0), stop=(k == c.D_T - 1))
                lg = tp.tile([P, c.E], F32, name="lg3", tag="rtE", bufs=6)
                nc.vector.tensor_tensor(out=lg, in0=ps_lg, in1=rb_bc,
                                        op=ALU.add)
                ind3 = tp.tile([P, c.E], F32, name="ind3", tag="rtE", bufs=6)
                probs3 = router_probs(lg, tp, tstat, out_ind=ind3)
                nc.tensor.matmul(ps_imp, probs3, ones_col, start=(j == 0),
                                 stop=(j == c.CH_T - 1))
                nc.tensor.matmul(ps_load, ind3, ones_col, start=(j == 0),
                                 stop=(j == c.CH_T - 1))

            aux_sb = tp.tile([c.E, 2], F32, name="aux_sb", tag="aux", bufs=4)
            nc.vector.tensor_copy(aux_sb[:, 0:1], ps_imp)
            nc.vector.tensor_copy(aux_sb[:, 1:2], ps_load)
            nc.sync.dma_start(out=cc_aux_in[:], in_=aux_sb)
            nc.gpsimd.collective_compute(
                "AllReduce", ALU.add, replica_groups=[list(range(c.NC))],
                ins=[cc_aux_in[:]], outs=[cc_aux_out[:]])
            ax = tp.tile([c.E, 2], F32, name="ax", tag="aux", bufs=4)
            nc.sync.dma_start(out=ax, in_=cc_aux_out[:])
            prod = tp.tile([c.E, 1], F32, name="prod", tag="aux", bufs=4)
            nc.vector.tensor_tensor(out=prod, in0=ax[:, 0:1], in1=ax[:, 1:2],
                                    op=ALU.mult)
            ps_f = tpsb.tile([1, 1], F32, name="ps_f", tag="psb")
            nc.tensor.matmul(ps_f, prod, ones_col[0:c.E, :], start=True,
                             stop=True)
            aux_fin = tp.tile([1, 1], F32, name="aux_fin", tag="aux", bufs=4)
            scale = float(c.E) / (float(c.NTOK) * float(c.NTOK * c.K))
            nc.scalar.activation(aux_fin, ps_f, AF.Copy, scale=scale)
            nc.sync.dma_start(out=outs["out_aux"][:], in_=aux_fin)


# ======================= host-side helpers =======================

def prepare_in_maps(inputs, cfg):
    """Full (unsharded) numpy inputs -> per-core in_maps."""
    c = cfg
    bf = ml_dtypes.bfloat16
    x = np.asarray(inputs["x"], np.float32)
    enc = np.asarray(inputs["encoder_output"], np.float32)
    per_batch = c.NC // c.B
    in_maps = []
    shared = {}
    for p in ("sa", "ca"):
        for m in ("wq", "wk", "wv", "wo"):
            shared[f"{p}_{m}"] = np.ascontiguousarray(
                np.asarray(inputs[f"{p}_{m}"], np.float32).astype(bf))
            shared[f"{p}_b{m[1]}"] = np.asarray(inputs[f"{p}_b{m[1]}"],
                                                np.float32)
    for n in ("n1", "n2", "n3"):
        shared[n + "_g"] = np.asarray(inputs[n + "_g"], np.float32)
        shared[n + "_b"] = np.asarray(inputs[n + "_b"], np.float32)
    shared["rw"] = np.asarray(inputs["r_w"], np.float32)
    shared["rb"] = np.asarray(inputs["r_b"], np.float32)
    e_w1 = np.asarray(inputs["e_w1"], np.float32)
    e_b1 = np.asarray(inputs["e_b1"], np.float32)
    e_w2 = np.asarray(inputs["e_w2"], np.float32)
    e_b2 = np.asarray(inputs["e_b2"], np.float32)
    for core in range(c.NC):
        b = core // per_batch
        q0 = (core % per_batch) * c.CHUNK
        m = dict(shared)
        m["xkv"] = np.ascontiguousarray(x[b].T.astype(bf))
        m["xq"] = np.ascontiguousarray(x[b, q0:q0 + c.CHUNK].T.astype(bf))
        m["x_tm"] = np.ascontiguousarray(x[b, q0:q0 + c.CHUNK])
        m["enc"] = np.ascontiguousarray(enc[b].T.astype(bf))
        m["w1"] = np.ascontiguousarray(e_w1[core].astype(bf))
        m["b1"] = np.ascontiguousarray(e_b1[core])
        m["w2"] = np.ascontiguousarray(e_w2[core].astype(bf))
        m["b2"] = np.ascontiguousarray(e_b2[core])
        sel = np.zeros((c.E,), np.float32)
        sel[core] = 1.0
        m["sel"] = sel
        in_maps.append(m)
    return in_maps


def input_specs(cfg):
    """name -> (shape, mybir dtype) for declaring dram tensors."""
    c = cfg
    sp = {
        "xkv": ([c.D, c.KV], BF16), "xq": ([c.D, c.CHUNK], BF16),
        "x_tm": ([c.CHUNK, c.D], F32), "enc": ([c.D, c.KV], BF16),
        "rw": ([c.D, c.E], F32), "rb": ([c.E], F32),
        "w1": ([c.D, c.F], BF16), "b1": ([c.F], F32),
        "w2": ([c.F, c.D], BF16), "b2": ([c.D], F32),
        "sel": ([c.E], F32),
    }
    for p in ("sa", "ca"):
        for m in ("wq", "wk", "wv", "wo"):
            sp[f"{p}_{m}"] = ([c.D, c.D], BF16)
            sp[f"{p}_b{m[1]}"] = ([c.D], F32)
    for n in ("n1", "n2", "n3"):
        sp[n + "_g"] = ([c.D], F32)
        sp[n + "_b"] = ([c.D], F32)
    return sp


def gather_outputs(results, cfg):
    c = cfg
    per_batch = c.NC // c.B
    x_full = np.zeros((c.B, c.S, c.D), np.float32)
    for core in range(c.NC):
        b = core // per_batch
        q0 = (core % per_batch) * c.CHUNK
        x_full[b, q0:q0 + c.CHUNK] = results[core]["out_x"]
    aux = np.float32(results[0]["out_aux"][0, 0])
    return x_full, aux




# ======================= public entry point =======================

_COMPILED = None
LAST_EXEC_NS = None


def _get_compiled():
    global _COMPILED
    if _COMPILED is None:
        from concourse import bacc
        cfg = Cfg()
        nc = bacc.Bacc("TRN2", target_bir_lowering=False, debug=False,
                       num_devices=cfg.NC)
        ins = {name: nc.dram_tensor(name, shape, dt,
                                    kind="ExternalInput").ap()
               for name, (shape, dt) in input_specs(cfg).items()}
        outs = {
            "out_x": nc.dram_tensor("out_x", [cfg.CHUNK, cfg.D], F32,
                                    kind="ExternalOutput").ap(),
            "out_aux": nc.dram_tensor("out_aux", [1, 1], F32,
                                      kind="ExternalOutput").ap(),
        }
        with tile.TileContext(nc) as tc:
            build_kernel(tc, outs, ins, cfg)
        nc.compile()
        _COMPILED = (nc, cfg)
    return _COMPILED


def kernel(**inputs):
    from concourse.bass_utils import run_bass_kernel_spmd
    nc, cfg = _get_compiled()
    in_maps = prepare_in_maps(inputs, cfg)
    res = run_bass_kernel_spmd(nc, in_maps, core_ids=list(range(cfg.NC)))
    kernel.last_exec_time_ns = getattr(res, "exec_time_ns", None)
    global LAST_EXEC_NS
    LAST_EXEC_NS = kernel.last_exec_time_ns
    return gather_outputs(res.results, cfg)


# revision 3
# speedup vs baseline: 1.1112x; 1.1112x over previous
"""Self-contained Trainium2 Bass kernel for nn_DecoderLayerWithMoE.

kernel(**inputs) takes the FULL unsharded inputs (as produced by
setup_inputs()) and returns (x, aux) matching reference(**inputs).
Work is distributed across 8 NeuronCores: data-parallel attention,
expert-parallel top-2-routed MoE (capacity-based token gather via
indirect DMA) with AllGather / ReduceScatter / AllReduce collectives.
"""

import sys
sys.path.insert(0, "/opt/trn_rl_repo")

"""Builder for the DecoderLayerWithMoE distributed Bass kernel.

Distribution over 8 cores:
  - Attention + LayerNorms: data-parallel. Core c handles batch b=c//(NC//B),
    query-token chunk (c % (NC//B)) of size CHUNK = B*S/NC. K/V computed
    redundantly from the full per-batch sequence (inputs replicated per batch).
  - MoE: expert-parallel dense. Core c owns expert e=c, computes its expert for
    ALL tokens, scales by the (zero-for-unselected) combine gate, and a
    ReduceScatter(add) returns each core its own token chunk.
  - Router runs in fp32 from exact (pre-AllGather) logits so top-2 decisions
    match the reference up to fp32 rounding.
  - Aux loss: per-core partial sums over its chunk + tiny AllReduce.

Layout conventions:
  fm = feature-major [D on partitions, tokens on free]  (matmul-friendly)
  tm = token-major  [tokens on partitions, D on free]   (LayerNorm-friendly)
"""

import numpy as np
import ml_dtypes

import concourse.bass as bass
import concourse.mybir as mybir
import concourse.tile as tile
from concourse.masks import make_identity, make_causal_mask
from concourse.tile_rust import add_dep_helper

F32 = mybir.dt.float32
BF16 = mybir.dt.bfloat16
AF = mybir.ActivationFunctionType
ALU = mybir.AluOpType
AX = mybir.AxisListType
I32 = mybir.dt.int32
P = 128


def part_bcast(ap, p):
    """Broadcast a 1-D dram AP along a new leading partition dim."""
    return bass.AP(tensor=ap.tensor, offset=ap.offset,
                   ap=[[0, p]] + [list(x) for x in ap.ap])


class Cfg:
    def __init__(self, D=1024, H=16, F=4096, E=8, K=2, B=2, S=2048, NC=8,
                 routed_cap=None):
        self.D, self.H, self.F, self.E, self.K, self.B, self.S, self.NC = (
            D, H, F, E, K, B, S, NC)
        self.DH = D // H
        self.NTOK = B * S
        self.CHUNK = self.NTOK // NC     # tokens per core
        self.KV = S                      # kv length per batch
        self.D_T = D // P
        self.F_T = F // P
        self.CH_T = self.CHUNK // P
        self.KV_T = self.KV // P
        self.KV_N = max(1, self.KV // 512)   # 512-wide free slices over KV
        self.D_N = max(1, D // 512)          # 512-wide free slices over D
        self.HPG = 512 // self.DH            # heads per group
        self.NG = self.H // self.HPG         # head groups
        assert D % P == 0 and F % P == 0 and self.CHUNK % P == 0
        assert self.DH == 64 and E == 8 and K == 2
        assert self.CHUNK <= 512 and self.KV % 512 == 0 and D % 512 == 0
        assert self.H * self.DH == D
        self.routed_cap = routed_cap
        if routed_cap is not None:
            assert routed_cap % 512 == 0 and routed_cap <= self.NTOK


def build_kernel(tc, outs, ins, cfg):
    nc = tc.nc
    c = cfg

    with tc.tile_pool(name="const", bufs=1) as const, \
         tc.tile_pool(name="x2res", bufs=c.CH_T) as x2res_pool, \
         tc.tile_pool(name="dram", bufs=1, space="DRAM") as dram:

        # ---------------- constants ----------------
        ident_f = const.tile([P, P], F32, name="ident_f")
        make_identity(nc, ident_f)
        ident_b = const.tile([P, P], BF16, name="ident_b")
        make_identity(nc, ident_b)
        ones_row = const.tile([1, P], F32, name="ones_row")
        nc.vector.memset(ones_row, 1.0)
        ones_col = const.tile([P, 1], F32, name="ones_col")
        nc.vector.memset(ones_col, 1.0)
        rb_bc = const.tile([P, c.E], F32, name="rb_bc")
        nc.gpsimd.dma_start(out=rb_bc, in_=part_bcast(ins["rb"], P))
        sel_bc = const.tile([P, c.E], F32, name="sel_bc")
        nc.gpsimd.dma_start(out=sel_bc, in_=part_bcast(ins["sel"], P))
        rw_sb = []
        for k in range(c.D_T):
            t = const.tile([P, c.E], F32, name=f"rw_sb{k}")
            nc.sync.dma_start(out=t, in_=ins["rw"][k * P:(k + 1) * P, :])
            rw_sb.append(t)
        if c.routed_cap:
            utri = const.tile([P, P], F32, name="utri")
            make_causal_mask(nc, utri, mask_val=1.0)  # 1 iff row<col
            iota_col = const.tile([P, 1], I32, name="iota_col")
            nc.gpsimd.iota(iota_col, pattern=[[0, 1]], base=0,
                           channel_multiplier=1)

        # ---------------- collective DRAM buffers ----------------
        if c.routed_cap:
            cc_xtm_in = dram.tile([c.CHUNK, c.D], BF16, name="cc_xtm_in")
            cc_xtm_out = dram.tile([c.NTOK, c.D], BF16, name="cc_xtm_out",
                                   addr_space="Shared")
            cmb_dram = dram.tile([c.NTOK, 1], F32, name="cmb_dram")
            slots_dram = dram.tile([c.routed_cap, 1], I32, name="slots_dram")
        else:
            cc_ag_in = dram.tile([c.D, c.CHUNK], BF16, name="cc_ag_in")
            cc_ag_out = dram.tile([c.NC * c.D, c.CHUNK], BF16,
                                  name="cc_ag_out", addr_space="Shared")
        cc_lg_in = dram.tile([c.CHUNK, c.E], F32, name="cc_lg_in")
        cc_lg_out = dram.tile([c.NC * c.CHUNK, c.E], F32, name="cc_lg_out",
                              addr_space="Shared")
        cc_y_in = dram.tile([c.NTOK, c.D], BF16, name="cc_y_in")
        cc_y_out = dram.tile([c.CHUNK, c.D], BF16, name="cc_y_out")
        cc_aux_in = dram.tile([c.E, 2], F32, name="cc_aux_in")
        cc_aux_out = dram.tile([c.E, 2], F32, name="cc_aux_out",
                               addr_space="Shared")

        # ---------------- shared helpers ----------------
        def bcast_row(pl, name, width, tag="lnp", bufs=3, dt=F32):
            t = pl.tile([P, width], dt, name=f"br_{name}", tag=tag, bufs=bufs)
            nc.gpsimd.dma_start(out=t, in_=part_bcast(ins[name], P))
            return t

        def layernorm(src, dst, g_bc, b_bc, sqpool, stat):
            """token-major LN over free dim D (in-place safe: dst != src)."""
            sum_x = stat.tile([P, 1], F32, name="ln_sumx", tag="st")
            nc.vector.reduce_sum(sum_x, src, axis=AX.X)
            sqd = sqpool.tile([P, c.D], BF16, name="ln_sqd", tag="sq", bufs=2)
            sum_sq = stat.tile([P, 1], F32, name="ln_sumsq", tag="st")
            nc.vector.scalar_tensor_tensor(
                out=sqd, in0=src, scalar=1.0, in1=src,
                op0=ALU.mult, op1=ALU.mult, accum_out=sum_sq)
            mean = stat.tile([P, 1], F32, name="ln_mean", tag="st")
            nc.vector.tensor_scalar(out=mean, in0=sum_x, scalar1=1.0 / c.D,
                                    scalar2=None, op0=ALU.mult)
            msq = stat.tile([P, 1], F32, name="ln_msq", tag="st")
            nc.vector.tensor_tensor(out=msq, in0=mean, in1=mean, op=ALU.mult)
            # var + eps = sum_sq/D - mean^2 + eps
            var = stat.tile([P, 1], F32, name="ln_var", tag="st")
            nc.vector.tensor_scalar(out=var, in0=sum_sq, scalar1=1.0 / c.D,
                                    scalar2=1e-5, op0=ALU.mult, op1=ALU.add)
            vareps = stat.tile([P, 1], F32, name="ln_vareps", tag="st")
            nc.vector.tensor_tensor(out=vareps, in0=var, in1=msq,
                                    op=ALU.subtract)
            std = stat.tile([P, 1], F32, name="ln_std", tag="st")
            nc.scalar.sqrt(std, vareps)
            rstd = stat.tile([P, 1], F32, name="ln_rstd", tag="st")
            nc.vector.reciprocal(rstd, std)
            nc.vector.tensor_scalar(out=dst, in0=src, scalar1=mean,
                                    scalar2=rstd, op0=ALU.subtract,
                                    op1=ALU.mult)
            nc.vector.tensor_tensor(out=dst, in0=dst, in1=g_bc, op=ALU.mult)
            nc.vector.tensor_tensor(out=dst, in0=dst, in1=b_bc, op=ALU.add)

        def router_probs(lg_ap, scr, stat, out_ind=None, out_cmb=None):
            """softmax over E + top-2 machinery from exact f32 logits."""
            m1a = stat.tile([P, 1], F32, name="rt_m1a", tag="st")
            nc.vector.reduce_max(m1a, lg_ap, axis=AX.X)
            negm = stat.tile([P, 1], F32, name="rt_negm", tag="st")
            nc.vector.tensor_scalar(out=negm, in0=m1a, scalar1=-1.0,
                                    scalar2=None, op0=ALU.mult)
            et = scr.tile([P, c.E], F32, name="rt_exp", tag="rtE", bufs=6)
            den = stat.tile([P, 1], F32, name="rt_den", tag="st")
            nc.scalar.activation(et, lg_ap, AF.Exp, bias=negm, accum_out=den)
            rd = stat.tile([P, 1], F32, name="rt_rd", tag="st")
            nc.vector.reciprocal(rd, den)
            probs = scr.tile([P, c.E], F32, name="rt_probs", tag="rtE", bufs=6)
            nc.vector.tensor_scalar(out=probs, in0=et, scalar1=rd,
                                    scalar2=None, op0=ALU.mult)
            m1 = stat.tile([P, 1], F32, name="rt_m1", tag="st")
            nc.vector.reduce_max(m1, probs, axis=AX.X)
            lt = scr.tile([P, c.E], F32, name="rt_lt", tag="rtE", bufs=6)
            nc.vector.tensor_scalar(out=lt, in0=probs, scalar1=m1,
                                    scalar2=None, op0=ALU.is_lt)
            pm = scr.tile([P, c.E], F32, name="rt_pm", tag="rtE", bufs=6)
            nc.vector.tensor_tensor(out=pm, in0=probs, in1=lt, op=ALU.mult)
            m2 = stat.tile([P, 1], F32, name="rt_m2", tag="st")
            nc.vector.reduce_max(m2, pm, axis=AX.X)
            if out_ind is not None:
                nc.vector.tensor_scalar(out=out_ind, in0=probs, scalar1=m2,
                                        scalar2=None, op0=ALU.is_ge)
            if out_cmb is not None:
                s12 = stat.tile([P, 1], F32, name="rt_s12", tag="st")
                nc.vector.tensor_tensor(out=s12, in0=m1, in1=m2, op=ALU.add)
                gsc = stat.tile([P, 1], F32, name="rt_gsc", tag="st")
                nc.vector.reciprocal(gsc, s12)
                pse = scr.tile([P, c.E], F32, name="rt_pse", tag="rtE", bufs=6)
                nc.vector.tensor_tensor(out=pse, in0=probs, in1=sel_bc,
                                        op=ALU.mult)
                pe = stat.tile([P, 1], F32, name="rt_pe", tag="st")
                nc.vector.reduce_sum(pe, pse, axis=AX.X)
                ind = stat.tile([P, 1], F32, name="rt_ind", tag="st")
                nc.vector.tensor_scalar(out=ind, in0=pe, scalar1=m2,
                                        scalar2=None, op0=ALU.is_ge)
                t1 = stat.tile([P, 1], F32, name="rt_t1", tag="st")
                nc.vector.tensor_tensor(out=t1, in0=pe, in1=gsc, op=ALU.mult)
                nc.vector.tensor_tensor(out=out_cmb, in0=t1, in1=ind,
                                        op=ALU.mult)
            return probs

        # ================= phases A + B: attention =================
        with tc.tile_pool(name="ab_big", bufs=1) as abp, \
             tc.tile_pool(name="ab_stat", bufs=24) as stat, \
             tc.tile_pool(name="ab_ps", bufs=4, space="PSUM") as ps_big, \
             tc.tile_pool(name="ab_psb", bufs=2, space="PSUM") as ps_small:

            def T(shape, dt, name, tag, bufs):
                return abp.tile(shape, dt, name=name, tag=tag, bufs=bufs)

            def bias_cols(name):
                t = abp.tile([P, c.D_T], F32, name=f"bc_{name}", tag="bcol",
                             bufs=8)
                nc.sync.dma_start(out=t,
                                  in_=ins[name].rearrange("(t p) -> p t", p=P))
                return t

            def attention(pfx, q_src, kv_dram, resid_in, ln_g, ln_b,
                          want_logits):
                tc.no_sync_barrier()
                EH = c.DH + 1
                bq, bk, bo = (bias_cols(pfx + "_bq"), bias_cols(pfx + "_bk"),
                              bias_cols(pfx + "_bo"))
                bv_bc = bcast_row(abp, pfx + "_bv", c.D)
                g_bc = bcast_row(abp, ln_g, c.D)
                b_bc = bcast_row(abp, ln_b, c.D)

                xt = []
                for k in range(c.D_T):
                    t = T([P, c.KV], BF16, f"{pfx}_xt{k}", "xt", c.D_T)
                    nc.sync.dma_start(out=t, in_=kv_dram[k * P:(k + 1) * P, :])
                    xt.append(t)

                def load_w_full(wname):
                    w = []
                    for k in range(c.D_T):
                        t = T([P, c.D], BF16, f"{wname}{k}", "wbig", c.D_T)
                        nc.sync.dma_start(out=t,
                                          in_=ins[wname][k * P:(k + 1) * P, :])
                        w.append(t)
                    return w

                # Q projection (fm)
                wq = load_w_full(pfx + "_wq")
                q_sb = []
                for m in range(c.D_T):
                    ps = ps_big.tile([P, c.CHUNK], F32, name="ps_q", tag="ps")
                    for k in range(c.D_T):
                        nc.tensor.matmul(ps, wq[k][:, m * P:(m + 1) * P],
                                         q_src[k], start=(k == 0),
                                         stop=(k == c.D_T - 1))
                    t = T([P, c.CHUNK], BF16, f"{pfx}_q{m}", "q", c.D_T)
                    nc.scalar.activation(t, ps, AF.Identity,
                                         bias=bq[:, m:m + 1])
                    q_sb.append(t)

                o_sb = [T([P, c.CHUNK], BF16, f"{pfx}_o{m}", "o", c.D_T)
                        for m in range(c.D_T)]

                for g in range(c.NG):
                    if g:
                        tc.no_sync_barrier()
                    gsl = slice(g * 512, (g + 1) * 512)
                    # K for this head group (fm rows g*512 .. +512)
                    wk_g = []
                    for k in range(c.D_T):
                        t = T([P, 512], BF16, f"{pfx}_wk{g}_{k}", "wsm",
                              c.D_T + 1)
                        nc.sync.dma_start(out=t,
                                          in_=ins[pfx + "_wk"][k * P:(k + 1) * P,
                                                               gsl])
                        wk_g.append(t)
                    k_g = []
                    for mi in range(512 // P):
                        t = T([P, c.KV], BF16, f"{pfx}_k{g}_{mi}", "k", 5)
                        for ns in range(c.KV_N):
                            nsl = slice(ns * 512, (ns + 1) * 512)
                            ps = ps_big.tile([P, 512], F32, name="ps_k",
                                             tag="ps")
                            for k in range(c.D_T):
                                nc.tensor.matmul(ps, wk_g[k][:, mi * P:(mi + 1) * P],
                                                 xt[k][:, nsl], start=(k == 0),
                                                 stop=(k == c.D_T - 1))
                            nc.scalar.activation(
                                t[:, nsl], ps, AF.Identity,
                                bias=bk[:, (g * 512 // P) + mi:
                                        (g * 512 // P) + mi + 1])
                        k_g.append(t)
                    # V for this head group (tm, interleaved ones col)
                    wv_g = []
                    for k in range(c.D_T):
                        t = T([P, 512], BF16, f"{pfx}_wv{g}_{k}", "wsm",
                              c.D_T + 1)
                        nc.sync.dma_start(out=t,
                                          in_=ins[pfx + "_wv"][k * P:(k + 1) * P,
                                                               gsl])
                        wv_g.append(t)
                    v_g = []
                    for tt in range(c.KV_T):
                        t = T([P, c.HPG * EH], BF16, f"{pfx}_v{g}_{tt}", "v",
                              c.KV_T + 1)
                        v3 = t.rearrange("p (h e) -> p h e", e=EH)
                        nc.vector.memset(v3[:, :, c.DH:EH], 1.0)
                        ps = ps_big.tile([P, 512], F32, name="ps_v", tag="ps")
                        for k in range(c.D_T):
                            nc.tensor.matmul(ps, xt[k][:, tt * P:(tt + 1) * P],
                                             wv_g[k], start=(k == 0),
                                             stop=(k == c.D_T - 1))
                        view = v3[:, :, 0:c.DH]
                        nc.scalar.activation(view, ps, AF.Identity)
                        bvv = bv_bc[:, gsl].rearrange("p (h d) -> p h d",
                                                      d=c.DH)
                        nc.vector.tensor_tensor(out=view, in0=view, in1=bvv,
                                                op=ALU.add)
                        v_g.append(t)
                    # heads in this group
                    for hl in range(c.HPG):
                        h = g * c.HPG + hl
                        ki, ro = (hl * c.DH) // P, (hl * c.DH) % P
                        q_i, q_ro = (h * c.DH) // P, (h * c.DH) % P
                        q_ap = q_sb[q_i][q_ro:q_ro + c.DH, :]
                        ps_o = ps_big.tile([c.DH + 1, c.CHUNK], F32,
                                           name="ps_o", tag="ps")
                        av_insts = []
                        for kt in range(c.KV_T):
                            ps_s = ps_big.tile([P, c.CHUNK], F32, name="ps_s",
                                               tag="ps")
                            sc = nc.tensor.matmul(
                                ps_s, k_g[ki][ro:ro + c.DH, kt * P:(kt + 1) * P],
                                q_ap, start=True, stop=True)
                            if kt >= 2:
                                # keep scores <=2 ahead of AV so pt/ps slots
                                # can't wedge the pipeline
                                add_dep_helper(sc.ins, av_insts[kt - 2].ins,
                                               sync=False, reason="pt pacing")
                            ptt = T([P, c.CHUNK], BF16, "pt", "pt", 4)
                            nc.scalar.activation(ptt, ps_s, AF.Exp,
                                                 scale=float(1.0 / np.sqrt(c.DH)))
                            av = nc.tensor.matmul(
                                ps_o, v_g[kt][:, hl * EH:(hl + 1) * EH],
                                ptt, start=(kt == 0), stop=(kt == c.KV_T - 1))
                            av_insts.append(av)
                        recip = stat.tile([1, c.CHUNK], F32, name="att_recip",
                                          tag="arec", bufs=2)
                        nc.vector.reciprocal(recip, ps_o[c.DH:c.DH + 1, :])
                        ps_b = ps_small.tile([c.DH, c.CHUNK], F32, name="ps_b",
                                             tag="psb")
                        nc.tensor.matmul(ps_b, ones_row[0:1, 0:c.DH], recip,
                                         start=True, stop=True)
                        rbc = T([c.DH, c.CHUNK], BF16, "rbc", "abc", 3)
                        nc.scalar.copy(rbc, ps_b)
                        oi, oro = (h * c.DH) // P, (h * c.DH) % P
                        nc.vector.tensor_tensor(
                            out=o_sb[oi][oro:oro + c.DH, :],
                            in0=ps_o[0:c.DH, :], in1=rbc, op=ALU.mult)

                # out projection (fm, bf16) + transpose + residual + LN (tm)
                wo = load_w_full(pfx + "_wo")
                ao = []
                for m in range(c.D_T):
                    ps = ps_big.tile([P, c.CHUNK], F32, name="ps_ao", tag="ps")
                    for k in range(c.D_T):
                        nc.tensor.matmul(ps, wo[k][:, m * P:(m + 1) * P],
                                         o_sb[k], start=(k == 0),
                                         stop=(k == c.D_T - 1))
                    t = T([P, c.CHUNK], BF16, f"{pfx}_ao{m}", "ao", c.D_T)
                    nc.scalar.activation(t, ps, AF.Identity,
                                         bias=bo[:, m:m + 1])
                    ao.append(t)

                x_next = []
                for j in range(c.CH_T):
                    xpre = T([P, c.D], F32, f"{pfx}_xpre{j}", "resid", 5)
                    for m in range(c.D_T):
                        ps_t = ps_small.tile([P, P], BF16, name="ps_t",
                                             tag="psb")
                        nc.tensor.transpose(ps_t, ao[m][:, j * P:(j + 1) * P],
                                            ident_b)
                        nc.vector.tensor_tensor(
                            out=xpre[:, m * P:(m + 1) * P], in0=ps_t,
                            in1=resid_in[j][:, m * P:(m + 1) * P], op=ALU.add)
                    if want_logits:
                        dst = x2res_pool.tile([P, c.D], F32, name=f"x2tm{j}",
                                              tag="x2")
                    else:
                        dst = T([P, c.D], F32, f"{pfx}_xn{j}", "resid", 5)
                    layernorm(xpre, dst, g_bc, b_bc, abp, stat)
                    x_next.append(dst)

                # transpose x_next -> fm bf16 (+ f32 logits if requested)
                x_fm_bf = [T([P, c.CHUNK], BF16, f"{pfx}_xfmb{m}", "xfm",
                             c.D_T) for m in range(c.D_T)]
                for j in range(c.CH_T):
                    tmp32 = []
                    for m in range(c.D_T):
                        ps_t = ps_small.tile([P, P], F32, name="ps_t2",
                                             tag="psb")
                        nc.tensor.transpose(ps_t,
                                            x_next[j][:, m * P:(m + 1) * P],
                                            ident_f)
                        nc.scalar.copy(x_fm_bf[m][:, j * P:(j + 1) * P], ps_t)
                        if want_logits:
                            tf = T([P, P], F32, f"t32_{m}", "t32", c.D_T + 1)
                            nc.vector.tensor_copy(tf, ps_t)
                            tmp32.append(tf)
                    if want_logits:
                        ps_lg = ps_small.tile([P, c.E], F32, name="ps_lg",
                                              tag="psb")
                        for k in range(c.D_T):
                            nc.tensor.matmul(ps_lg, tmp32[k], rw_sb[k],
                                             start=(k == 0),
                                             stop=(k == c.D_T - 1))
                        lg = abp.tile([P, c.E], F32, name="lg_sb", tag="rtE",
                                      bufs=6)
                        nc.vector.tensor_tensor(out=lg, in0=ps_lg, in1=rb_bc,
                                                op=ALU.add)
                        nc.sync.dma_start(out=cc_lg_in[j * P:(j + 1) * P, :],
                                          in_=lg)
                return x_next, x_fm_bf

            # ---- run the two attention phases ----
            xq = []
            for k in range(c.D_T):
                t = abp.tile([P, c.CHUNK], BF16, name=f"xq{k}", tag="xq",
                             bufs=c.D_T)
                nc.sync.dma_start(out=t, in_=ins["xq"][k * P:(k + 1) * P, :])
                xq.append(t)
            x_tm = []
            for j in range(c.CH_T):
                t = abp.tile([P, c.D], F32, name=f"xtm{j}", tag="resid",
                             bufs=5)
                nc.sync.dma_start(out=t, in_=ins["x_tm"][j * P:(j + 1) * P, :])
                x_tm.append(t)

            x1_tm, x1_fm = attention("sa", xq, ins["xkv"], x_tm,
                                     "n1_g", "n1_b", False)
            x2_tm, x2_fm = attention("ca", x1_fm, ins["enc"], x1_tm,
                                     "n2_g", "n2_b", True)

            if c.routed_cap:
                for j in range(c.CH_T):
                    xtb = abp.tile([P, c.D], BF16, name=f"xtb{j}", tag="sq",
                                   bufs=2)
                    nc.vector.tensor_copy(xtb, x2_tm[j])
                    nc.sync.dma_start(out=cc_xtm_in[j * P:(j + 1) * P, :],
                                      in_=xtb)
            else:
                for m in range(c.D_T):
                    nc.sync.dma_start(out=cc_ag_in[m * P:(m + 1) * P, :],
                                      in_=x2_fm[m])

        if c.routed_cap:
            nc.gpsimd.collective_compute(
                "AllGather", ALU.bypass, replica_groups=[list(range(c.NC))],
                ins=[cc_xtm_in[:]], outs=[cc_xtm_out[:]])
        else:
            nc.gpsimd.collective_compute(
                "AllGather", ALU.bypass, replica_groups=[list(range(c.NC))],
                ins=[cc_ag_in[:]], outs=[cc_ag_out[:]])
        nc.gpsimd.collective_compute(
            "AllGather", ALU.bypass, replica_groups=[list(range(c.NC))],
            ins=[cc_lg_in[:]], outs=[cc_lg_out[:]])

        # ================= phase C: MoE (dense expert-parallel) =============
        with tc.tile_pool(name="moe", bufs=1) as mp, \
             tc.tile_pool(name="moe_stat", bufs=24) as mstat, \
             tc.tile_pool(name="moe_ps", bufs=4, space="PSUM") as mps, \
             tc.tile_pool(name="moe_psb", bufs=2, space="PSUM") as mpsb:

            w1_sb = []
            for k in range(c.D_T):
                t = mp.tile([P, c.F], BF16, name=f"w1sb{k}", tag="w1",
                            bufs=c.D_T)
                nc.sync.dma_start(out=t, in_=ins["w1"][k * P:(k + 1) * P, :])
                w1_sb.append(t)
            w2_sb = []
            for f in range(c.F_T):
                t = mp.tile([P, c.D], BF16, name=f"w2sb{f}", tag="w2",
                            bufs=c.F_T)
                nc.sync.dma_start(out=t, in_=ins["w2"][f * P:(f + 1) * P, :])
                w2_sb.append(t)
            b1_sb = mp.tile([P, c.F_T], F32, name="b1_sb", tag="b1", bufs=1)
            nc.sync.dma_start(out=b1_sb,
                              in_=ins["b1"].rearrange("(t p) -> p t", p=P))
            b2_bc = bcast_row(mp, "b2", c.D, tag="lnp", bufs=1, dt=BF16)

            def ffn_block(xsel, cmbg, scatter_idx):
                """relu(x@w1+b1) @ w2 + b2, gated, scattered to cc_y_in."""
                h_bf = []
                for f in range(c.F_T):
                    ps = mps.tile([P, 512], F32, name="ps_h", tag="ps")
                    for k in range(c.D_T):
                        nc.tensor.matmul(ps, w1_sb[k][:, f * P:(f + 1) * P],
                                         xsel[k], start=(k == 0),
                                         stop=(k == c.D_T - 1))
                    t = mp.tile([P, 512], BF16, name=f"h{f}", tag="h",
                                bufs=c.F_T)
                    nc.scalar.activation(t, ps, AF.Relu,
                                         bias=b1_sb[:, f:f + 1])
                    h_bf.append(t)
                for tt in range(4):
                    yrow = mp.tile([P, c.D], BF16, name="yrow", tag="y", bufs=2)
                    for dn in range(c.D_N):
                        ps = mps.tile([P, 512], F32, name="ps_y", tag="ps")
                        for f in range(c.F_T):
                            nc.tensor.matmul(
                                ps, h_bf[f][:, tt * P:(tt + 1) * P],
                                w2_sb[f][:, dn * 512:(dn + 1) * 512],
                                start=(f == 0), stop=(f == c.F_T - 1))
                        nc.vector.tensor_tensor(
                            out=yrow[:, dn * 512:(dn + 1) * 512], in0=ps,
                            in1=b2_bc[:, dn * 512:(dn + 1) * 512], op=ALU.add)
                    y2 = mp.tile([P, c.D], BF16, name="y2", tag="y2", bufs=2)
                    nc.vector.tensor_scalar(out=y2, in0=yrow,
                                            scalar1=cmbg[tt], scalar2=None,
                                            op0=ALU.mult)
                    nc.gpsimd.indirect_dma_start(
                        out=cc_y_in[:],
                        out_offset=bass.IndirectOffsetOnAxis(
                            ap=scatter_idx[tt][:, 0:1], axis=0),
                        in_=y2[:], in_offset=None,
                        bounds_check=c.NTOK - 1, oob_is_err=False)

            if c.routed_cap:
                CAP = c.routed_cap
                GRP = c.NTOK // P
                # zero the bf16 y buffer (RS sums all ranks)
                zt = mp.tile([P, 512], BF16, name="zt", tag="zt", bufs=1)
                nc.vector.memset(zt, 0.0)
                for r in range(GRP):
                    for dn in range(c.D_N):
                        nc.sync.dma_start(
                            out=cc_y_in[r * P:(r + 1) * P,
                                        dn * 512:(dn + 1) * 512], in_=zt)
                # slots table starts as all-dropped
                padt = mp.tile([P, 1], I32, name="padt", tag="small", bufs=16)
                nc.vector.memset(padt, 2 * c.NTOK)
                for s in range(CAP // P):
                    nc.sync.dma_start(out=slots_dram[s * P:(s + 1) * P, :],
                                      in_=padt)
                # routing for all tokens; flags[p, g] = token g*128+p selected
                flags = mp.tile([P, GRP], F32, name="flags", tag="flags",
                                bufs=1)
                for g in range(GRP):
                    row = g * P
                    lg = mp.tile([P, c.E], F32, name="moe_lg", tag="rtE",
                                 bufs=6)
                    nc.sync.dma_start(out=lg, in_=cc_lg_out[row:row + P, :])
                    cm = mp.tile([P, 1], F32, name="cmr", tag="cmb",
                                 bufs=c.CH_T + 4)
                    router_probs(lg, mp, mstat, out_cmb=cm)
                    nc.sync.dma_start(out=cmb_dram[row:row + P, :], in_=cm)
                    nc.vector.tensor_scalar(out=flags[:, g:g + 1], in0=cm,
                                            scalar1=0.0, scalar2=None,
                                            op0=ALU.is_gt)
                # group counts -> exclusive group offsets
                ps_cnt = mpsb.tile([GRP, 1], F32, name="ps_cnt", tag="psb")
                nc.tensor.matmul(ps_cnt, flags, ones_col, start=True,
                                 stop=True)
                cnt_sb = mp.tile([GRP, 1], F32, name="cnt_sb", tag="small",
                                 bufs=16)
                nc.vector.tensor_copy(cnt_sb, ps_cnt)
                ps_cr = mpsb.tile([1, GRP], F32, name="ps_cr", tag="psb")
                nc.tensor.matmul(ps_cr, cnt_sb, ident_f[0:GRP, 0:GRP],
                                 start=True, stop=True)
                cnt_row = mp.tile([1, GRP], F32, name="cnt_row", tag="small",
                                  bufs=16)
                nc.vector.tensor_copy(cnt_row, ps_cr)
                zrow = mp.tile([1, GRP], F32, name="zrow", tag="small",
                               bufs=16)
                nc.vector.memset(zrow, 0.0)
                incl = mp.tile([1, GRP], F32, name="incl", tag="small",
                               bufs=16)
                nc.vector.tensor_tensor_scan(out=incl, data0=cnt_row,
                                             data1=zrow, initial=0.0,
                                             op0=ALU.add, op1=ALU.add)
                goff = mp.tile([1, GRP], F32, name="goff", tag="small",
                               bufs=16)
                nc.vector.tensor_tensor(out=goff, in0=incl, in1=cnt_row,
                                        op=ALU.subtract)
                # per-token slot = within-group exclusive prefix + group offset
                ps_pos = mps.tile([P, GRP], F32, name="ps_pos", tag="ps")
                nc.tensor.matmul(ps_pos, utri, flags, start=True, stop=False)
                nc.tensor.matmul(ps_pos, ones_row[0:1, :], goff, start=False,
                                 stop=True)
                notf = mp.tile([P, GRP], F32, name="notf", tag="notf",
                               bufs=1)
                nc.vector.tensor_scalar(out=notf, in0=flags, scalar1=0.5,
                                        scalar2=float(3 * c.NTOK),
                                        op0=ALU.is_lt, op1=ALU.mult)
                posm = mp.tile([P, GRP], F32, name="posm", tag="posm",
                               bufs=1)
                nc.vector.tensor_tensor(out=posm, in0=ps_pos, in1=notf,
                                        op=ALU.add)
                for g in range(GRP):
                    pcol = mp.tile([P, 1], I32, name="pcol", tag="small",
                                   bufs=16)
                    nc.vector.tensor_copy(pcol, posm[:, g:g + 1])
                    tcol = mp.tile([P, 1], I32, name="tcol", tag="small",
                                   bufs=16)
                    nc.vector.tensor_scalar(out=tcol, in0=iota_col,
                                            scalar1=g * P, scalar2=None,
                                            op0=ALU.add)
                    nc.gpsimd.indirect_dma_start(
                        out=slots_dram[:],
                        out_offset=bass.IndirectOffsetOnAxis(
                            ap=pcol[:, 0:1], axis=0),
                        in_=tcol[:, 0:1], in_offset=None,
                        bounds_check=CAP - 1, oob_is_err=False)
                # gather + FFN + scatter per 512-token block of slots
                for sb in range(CAP // 512):
                    if sb:
                        tc.no_sync_barrier()
                    xsel = [mp.tile([P, 512], BF16, name=f"xb{k}", tag="xb",
                                    bufs=c.D_T + 1) for k in range(c.D_T)]
                    cmbg, idxs = [], []
                    for g4 in range(4):
                        s0 = sb * 512 + g4 * P
                        idx = mp.tile([P, 1], I32, name="idx", tag="small",
                                      bufs=16)
                        nc.sync.dma_start(out=idx,
                                          in_=slots_dram[s0:s0 + P, :])
                        idxc = mp.tile([P, 1], I32, name="idxc", tag="small",
                                       bufs=16)
                        nc.vector.tensor_scalar(out=idxc, in0=idx,
                                                scalar1=c.NTOK - 1,
                                                scalar2=None, op0=ALU.min)
                        xg = mp.tile([P, c.D], BF16, name="xg", tag="xg",
                                     bufs=3)
                        nc.gpsimd.indirect_dma_start(
                            out=xg[:], out_offset=None, in_=cc_xtm_out[:],
                            in_offset=bass.IndirectOffsetOnAxis(
                                ap=idxc[:, 0:1], axis=0))
                        cg = mp.tile([P, 1], F32, name="cg", tag="cmb",
                                     bufs=c.CH_T + 4)
                        nc.gpsimd.indirect_dma_start(
                            out=cg[:], out_offset=None, in_=cmb_dram[:],
                            in_offset=bass.IndirectOffsetOnAxis(
                                ap=idxc[:, 0:1], axis=0))
                        cmbg.append(cg)
                        idxs.append(idx)
                        for k in range(c.D_T):
                            ps_t = mpsb.tile([P, P], BF16, name="ps_tx",
                                             tag="psb")
                            nc.tensor.transpose(ps_t, xg[:, k * P:(k + 1) * P],
                                                ident_b)
                            nc.scalar.copy(xsel[k][:, g4 * P:(g4 + 1) * P],
                                           ps_t)
                    ffn_block(xsel, cmbg, idxs)
            else:
                for blk in range(c.NC):
                    if blk:
                        tc.no_sync_barrier()
                    xb = []
                    for k in range(c.D_T):
                        t = mp.tile([P, c.CHUNK], BF16, name=f"xb{k}",
                                    tag="xb", bufs=c.D_T + 2)
                        nc.sync.dma_start(
                            out=t, in_=cc_ag_out[blk * c.D + k * P:
                                                 blk * c.D + (k + 1) * P, :])
                        xb.append(t)
                    cmb = []
                    for tt in range(c.CH_T):
                        lg = mp.tile([P, c.E], F32, name="moe_lg", tag="rtE",
                                     bufs=6)
                        row = blk * c.CHUNK + tt * P
                        nc.sync.dma_start(out=lg,
                                          in_=cc_lg_out[row:row + P, :])
                        cm = mp.tile([P, 1], F32, name=f"cmb{tt}", tag="cmb",
                                     bufs=c.CH_T + 2)
                        router_probs(lg, mp, mstat, out_cmb=cm)
                        cmb.append(cm)
                    h_bf = []
                    for f in range(c.F_T):
                        ps = mps.tile([P, c.CHUNK], F32, name="ps_h", tag="ps")
                        for k in range(c.D_T):
                            nc.tensor.matmul(ps,
                                             w1_sb[k][:, f * P:(f + 1) * P],
                                             xb[k], start=(k == 0),
                                             stop=(k == c.D_T - 1))
                        t = mp.tile([P, c.CHUNK], BF16, name=f"h{f}", tag="h",
                                    bufs=c.F_T + 2)
                        nc.scalar.activation(t, ps, AF.Relu,
                                             bias=b1_sb[:, f:f + 1])
                        h_bf.append(t)
                    for tt in range(c.CH_T):
                        for dn in range(c.D_N):
                            ps = mps.tile([P, 512], F32, name="ps_y", tag="ps")
                            for f in range(c.F_T):
                                nc.tensor.matmul(
                                    ps, h_bf[f][:, tt * P:(tt + 1) * P],
                                    w2_sb[f][:, dn * 512:(dn + 1) * 512],
                                    start=(f == 0), stop=(f == c.F_T - 1))
                            yt = mp.tile([P, 512], F32, name="yt", tag="y",
                                         bufs=4)
                            nc.vector.tensor_tensor(
                                out=yt, in0=ps,
                                in1=b2_bc[:, dn * 512:(dn + 1) * 512],
                                op=ALU.add)
                            yt2 = mp.tile([P, 512], BF16, name="yt2",
                                          tag="y2", bufs=4)
                            nc.vector.tensor_scalar(out=yt2, in0=yt,
                                                    scalar1=cmb[tt],
                                                    scalar2=None,
                                                    op0=ALU.mult)
                            row = blk * c.CHUNK + tt * P
                            nc.sync.dma_start(
                                out=cc_y_in[row:row + P,
                                            dn * 512:(dn + 1) * 512],
                                in_=yt2)

            nc.gpsimd.collective_compute(
                "ReduceScatter", ALU.add, replica_groups=[list(range(c.NC))],
                ins=[cc_y_in[:]], outs=[cc_y_out[:]])

        # ================= tail: LN3 + aux loss =================
        with tc.tile_pool(name="tl", bufs=1) as tp, \
             tc.tile_pool(name="tl_stat", bufs=24) as tstat, \
             tc.tile_pool(name="tl_ps", bufs=2, space="PSUM") as tps, \
             tc.tile_pool(name="tl_psb", bufs=2, space="PSUM") as tpsb:

            g3_bc = bcast_row(tp, "n3_g", c.D, tag="lnp", bufs=2)
            b3_bc = bcast_row(tp, "n3_b", c.D, tag="lnp", bufs=2)

            x3_tm = []
            for j in range(c.CH_T):
                yj = tp.tile([P, c.D], BF16, name=f"yj{j}", tag="yj", bufs=4)
                nc.sync.dma_start(out=yj, in_=cc_y_out[j * P:(j + 1) * P, :])
                x3pre = tp.tile([P, c.D], F32, name=f"x3pre{j}", tag="resid",
                                bufs=4)
                nc.vector.tensor_tensor(out=x3pre, in0=yj, in1=x2_tm[j],
                                        op=ALU.add)
                x3 = tp.tile([P, c.D], F32, name=f"x3_{j}", tag="x3",
                             bufs=c.CH_T)
                layernorm(x3pre, x3, g3_bc, b3_bc, tp, tstat)
                nc.sync.dma_start(out=outs["out_x"][j * P:(j + 1) * P, :],
                                  in_=x3)
                x3_tm.append(x3)

            ps_imp = tps.tile([c.E, 1], F32, name="ps_imp", tag="acc")
            ps_load = tps.tile([c.E, 1], F32, name="ps_load", tag="acc")
            for j in range(c.CH_T):
                tmp32 = []
                for m in range(c.D_T):
                    ps_t = tpsb.tile([P, P], F32, name="ps_t3", tag="psb")
                    nc.tensor.transpose(ps_t, x3_tm[j][:, m * P:(m + 1) * P],
                                        ident_f)
                    tf = tp.tile([P, P], F32, name=f"t33_{m}", tag="t32",
                                 bufs=c.D_T + 2)
                    nc.vector.tensor_copy(tf, ps_t)
                    tmp32.append(tf)
                ps_lg = tpsb.tile([P, c.E], F32, name="ps_lg3", tag="psb")
                for k in range(c.D_T):
                    nc.tensor.matmul(ps_lg, tmp32[k], rw_sb[k],
                                     start=(k == 0), stop=(k == c.D_T - 1))
                lg = tp.tile([P, c.E], F32, name="lg3", tag="rtE", bufs=6)
                nc.vector.tensor_tensor(out=lg, in0=ps_lg, in1=rb_bc,
                                        op=ALU.add)
                ind3 = tp.tile([P, c.E], F32, name="ind3", tag="rtE", bufs=6)
                probs3 = router_probs(lg, tp, tstat, out_ind=ind3)
                nc.tensor.matmul(ps_imp, probs3, ones_col, start=(j == 0),
                                 stop=(j == c.CH_T - 1))
                nc.tensor.matmul(ps_load, ind3, ones_col, start=(j == 0),
                                 stop=(j == c.CH_T - 1))

            aux_sb = tp.tile([c.E, 2], F32, name="aux_sb", tag="aux", bufs=4)
            nc.vector.tensor_copy(aux_sb[:, 0:1], ps_imp)
            nc.vector.tensor_copy(aux_sb[:, 1:2], ps_load)
            nc.sync.dma_start(out=cc_aux_in[:], in_=aux_sb)
            nc.gpsimd.collective_compute(
                "AllReduce", ALU.add, replica_groups=[list(range(c.NC))],
                ins=[cc_aux_in[:]], outs=[cc_aux_out[:]])
            ax = tp.tile([c.E, 2], F32, name="ax", tag="aux", bufs=4)
            nc.sync.dma_start(out=ax, in_=cc_aux_out[:])
            prod = tp.tile([c.E, 1], F32, name="prod", tag="aux", bufs=4)
            nc.vector.tensor_tensor(out=prod, in0=ax[:, 0:1], in1=ax[:, 1:2],
                                    op=ALU.mult)
            ps_f = tpsb.tile([1, 1], F32, name="ps_f", tag="psb")
            nc.tensor.matmul(ps_f, prod, ones_col[0:c.E, :], start=True,
                             stop=True)
            aux_fin = tp.tile([1, 1], F32, name="aux_fin", tag="aux", bufs=4)
            scale = float(c.E) / (float(c.NTOK) * float(c.NTOK * c.K))
            nc.scalar.activation(aux_fin, ps_f, AF.Copy, scale=scale)
            nc.sync.dma_start(out=outs["out_aux"][:], in_=aux_fin)


# ======================= host-side helpers =======================

def prepare_in_maps(inputs, cfg):
    """Full (unsharded) numpy inputs -> per-core in_maps."""
    c = cfg
    bf = ml_dtypes.bfloat16
    x = np.asarray(inputs["x"], np.float32)
    enc = np.asarray(inputs["encoder_output"], np.float32)
    per_batch = c.NC // c.B
    in_maps = []
    shared = {}
    for p in ("sa", "ca"):
        for m in ("wq", "wk", "wv", "wo"):
            shared[f"{p}_{m}"] = np.ascontiguousarray(
                np.asarray(inputs[f"{p}_{m}"], np.float32).astype(bf))
            shared[f"{p}_b{m[1]}"] = np.asarray(inputs[f"{p}_b{m[1]}"],
                                                np.float32)
    for n in ("n1", "n2", "n3"):
        shared[n + "_g"] = np.asarray(inputs[n + "_g"], np.float32)
        shared[n + "_b"] = np.asarray(inputs[n + "_b"], np.float32)
    shared["rw"] = np.asarray(inputs["r_w"], np.float32)
    shared["rb"] = np.asarray(inputs["r_b"], np.float32)
    e_w1 = np.asarray(inputs["e_w1"], np.float32)
    e_b1 = np.asarray(inputs["e_b1"], np.float32)
    e_w2 = np.asarray(inputs["e_w2"], np.float32)
    e_b2 = np.asarray(inputs["e_b2"], np.float32)
    for core in range(c.NC):
        b = core // per_batch
        q0 = (core % per_batch) * c.CHUNK
        m = dict(shared)
        m["xkv"] = np.ascontiguousarray(x[b].T.astype(bf))
        m["xq"] = np.ascontiguousarray(x[b, q0:q0 + c.CHUNK].T.astype(bf))
        m["x_tm"] = np.ascontiguousarray(x[b, q0:q0 + c.CHUNK])
        m["enc"] = np.ascontiguousarray(enc[b].T.astype(bf))
        m["w1"] = np.ascontiguousarray(e_w1[core].astype(bf))
        m["b1"] = np.ascontiguousarray(e_b1[core])
        m["w2"] = np.ascontiguousarray(e_w2[core].astype(bf))
        m["b2"] = np.ascontiguousarray(e_b2[core])
        sel = np.zeros((c.E,), np.float32)
        sel[core] = 1.0
        m["sel"] = sel
        in_maps.append(m)
    return in_maps


def input_specs(cfg):
    """name -> (shape, mybir dtype) for declaring dram tensors."""
    c = cfg
    sp = {
        "xkv": ([c.D, c.KV], BF16), "xq": ([c.D, c.CHUNK], BF16),
        "x_tm": ([c.CHUNK, c.D], F32), "enc": ([c.D, c.KV], BF16),
        "rw": ([c.D, c.E], F32), "rb": ([c.E], F32),
        "w1": ([c.D, c.F], BF16), "b1": ([c.F], F32),
        "w2": ([c.F, c.D], BF16), "b2": ([c.D], F32),
        "sel": ([c.E], F32),
    }
    for p in ("sa", "ca"):
        for m in ("wq", "wk", "wv", "wo"):
            sp[f"{p}_{m}"] = ([c.D, c.D], BF16)
            sp[f"{p}_b{m[1]}"] = ([c.D], F32)
    for n in ("n1", "n2", "n3"):
        sp[n + "_g"] = ([c.D], F32)
        sp[n + "_b"] = ([c.D], F32)
    return sp


def gather_outputs(results, cfg):
    c = cfg
    per_batch = c.NC // c.B
    x_full = np.zeros((c.B, c.S, c.D), np.float32)
    for core in range(c.NC):
        b = core // per_batch
        q0 = (core % per_batch) * c.CHUNK
        x_full[b, q0:q0 + c.CHUNK] = results[core]["out_x"]
    aux = np.float32(results[0]["out_aux"][0, 0])
    return x_full, aux




# ======================= public entry point =======================

_COMPILED = None
LAST_EXEC_NS = None


def _get_compiled():
    global _COMPILED
    if _COMPILED is None:
        from concourse import bacc
        cfg = Cfg(routed_cap=1536)
        nc = bacc.Bacc("TRN2", target_bir_lowering=False, debug=False,
                       num_devices=cfg.NC)
        ins = {name: nc.dram_tensor(name, shape, dt,
                                    kind="ExternalInput").ap()
               for name, (shape, dt) in input_specs(cfg).items()}
        outs = {
            "out_x": nc.dram_tensor("out_x", [cfg.CHUNK, cfg.D], F32,
                                    kind="ExternalOutput").ap(),
            "out_aux": nc.dram_tensor("out_aux", [1, 1], F32,
                                      kind="ExternalOutput").ap(),
        }
        with tile.TileContext(nc) as tc:
            build_kernel(tc, outs, ins, cfg)
        nc.compile()
        _COMPILED = (nc, cfg)
    return _COMPILED


def kernel(**inputs):
    from concourse.bass_utils import run_bass_kernel_spmd
    nc, cfg = _get_compiled()
    in_maps = prepare_in_maps(inputs, cfg)
    res = run_bass_kernel_spmd(nc, in_maps, core_ids=list(range(cfg.NC)))
    kernel.last_exec_time_ns = getattr(res, "exec_time_ns", None)
    global LAST_EXEC_NS
    LAST_EXEC_NS = kernel.last_exec_time_ns
    return gather_outputs(res.results, cfg)


# revision 4
# speedup vs baseline: 1.1428x; 1.0285x over previous
"""Self-contained Trainium2 Bass kernel for nn_DecoderLayerWithMoE.

kernel(**inputs) takes the FULL unsharded inputs (as produced by
setup_inputs()) and returns (x, aux) matching reference(**inputs).
Work is distributed across 8 NeuronCores: data-parallel attention,
expert-parallel top-2-routed MoE (capacity-based token gather via
indirect DMA) with AllGather / ReduceScatter / AllReduce collectives.
"""

import sys
sys.path.insert(0, "/opt/trn_rl_repo")

"""Builder for the DecoderLayerWithMoE distributed Bass kernel.

Distribution over 8 cores:
  - Attention + LayerNorms: data-parallel. Core c handles batch b=c//(NC//B),
    query-token chunk (c % (NC//B)) of size CHUNK = B*S/NC. K/V computed
    redundantly from the full per-batch sequence (inputs replicated per batch).
  - MoE: expert-parallel dense. Core c owns expert e=c, computes its expert for
    ALL tokens, scales by the (zero-for-unselected) combine gate, and a
    ReduceScatter(add) returns each core its own token chunk.
  - Router runs in fp32 from exact (pre-AllGather) logits so top-2 decisions
    match the reference up to fp32 rounding.
  - Aux loss: per-core partial sums over its chunk + tiny AllReduce.

Layout conventions:
  fm = feature-major [D on partitions, tokens on free]  (matmul-friendly)
  tm = token-major  [tokens on partitions, D on free]   (LayerNorm-friendly)
"""

import numpy as np
import ml_dtypes

import concourse.bass as bass
import concourse.mybir as mybir
import concourse.tile as tile
from concourse.masks import make_identity, make_causal_mask
from concourse.tile_rust import add_dep_helper

F32 = mybir.dt.float32
BF16 = mybir.dt.bfloat16
AF = mybir.ActivationFunctionType
ALU = mybir.AluOpType
AX = mybir.AxisListType
I32 = mybir.dt.int32
P = 128


def part_bcast(ap, p):
    """Broadcast a 1-D dram AP along a new leading partition dim."""
    return bass.AP(tensor=ap.tensor, offset=ap.offset,
                   ap=[[0, p]] + [list(x) for x in ap.ap])


class Cfg:
    def __init__(self, D=1024, H=16, F=4096, E=8, K=2, B=2, S=2048, NC=8,
                 routed_cap=None):
        self.D, self.H, self.F, self.E, self.K, self.B, self.S, self.NC = (
            D, H, F, E, K, B, S, NC)
        self.DH = D // H
        self.NTOK = B * S
        self.CHUNK = self.NTOK // NC     # tokens per core
        self.KV = S                      # kv length per batch
        self.D_T = D // P
        self.F_T = F // P
        self.CH_T = self.CHUNK // P
        self.KV_T = self.KV // P
        self.KV_N = max(1, self.KV // 512)   # 512-wide free slices over KV
        self.D_N = max(1, D // 512)          # 512-wide free slices over D
        self.HPG = 512 // self.DH            # heads per group
        self.NG = self.H // self.HPG         # head groups
        assert D % P == 0 and F % P == 0 and self.CHUNK % P == 0
        assert self.DH == 64 and E == 8 and K == 2
        assert self.CHUNK <= 512 and self.KV % 512 == 0 and D % 512 == 0
        assert self.H * self.DH == D
        self.routed_cap = routed_cap
        if routed_cap is not None:
            assert routed_cap % 512 == 0 and routed_cap <= self.NTOK


def build_kernel(tc, outs, ins, cfg):
    nc = tc.nc
    c = cfg

    with tc.tile_pool(name="const", bufs=1) as const, \
         tc.tile_pool(name="dram", bufs=1, space="DRAM") as dram:

        # ---------------- constants ----------------
        ident_f = const.tile([P, P], F32, name="ident_f")
        make_identity(nc, ident_f)
        ident_b = const.tile([P, P], BF16, name="ident_b")
        make_identity(nc, ident_b)
        ones_row = const.tile([1, P], F32, name="ones_row")
        nc.vector.memset(ones_row, 1.0)
        ones_col = const.tile([P, 1], F32, name="ones_col")
        nc.vector.memset(ones_col, 1.0)
        rb_bc = const.tile([P, c.E], F32, name="rb_bc")
        nc.gpsimd.dma_start(out=rb_bc, in_=part_bcast(ins["rb"], P))
        sel_bc = const.tile([P, c.E], F32, name="sel_bc")
        nc.gpsimd.dma_start(out=sel_bc, in_=part_bcast(ins["sel"], P))
        rw_sb = []
        for k in range(c.D_T):
            t = const.tile([P, c.E], F32, name=f"rw_sb{k}")
            nc.sync.dma_start(out=t, in_=ins["rw"][k * P:(k + 1) * P, :])
            rw_sb.append(t)
        if c.routed_cap:
            utri = const.tile([P, P], F32, name="utri")
            make_causal_mask(nc, utri, mask_val=1.0)  # 1 iff row<col
            iota_col = const.tile([P, 1], I32, name="iota_col")
            nc.gpsimd.iota(iota_col, pattern=[[0, 1]], base=0,
                           channel_multiplier=1)

        # ---------------- collective DRAM buffers ----------------
        if c.routed_cap:
            cc_xtm_in = dram.tile([c.CHUNK, c.D], BF16, name="cc_xtm_in")
            cc_xtm_out = dram.tile([c.NTOK, c.D], BF16, name="cc_xtm_out",
                                   addr_space="Shared")
            cmb_dram = dram.tile([c.NTOK, 1], F32, name="cmb_dram")
            slots_dram = dram.tile([c.routed_cap, 1], I32, name="slots_dram")
        else:
            cc_ag_in = dram.tile([c.D, c.CHUNK], BF16, name="cc_ag_in")
            cc_ag_out = dram.tile([c.NC * c.D, c.CHUNK], BF16,
                                  name="cc_ag_out", addr_space="Shared")
        cc_lg_in = dram.tile([c.CHUNK, c.E], F32, name="cc_lg_in")
        cc_lg_out = dram.tile([c.NC * c.CHUNK, c.E], F32, name="cc_lg_out",
                              addr_space="Shared")
        cc_y_in = dram.tile([c.NTOK, c.D], BF16, name="cc_y_in")
        cc_y_out = dram.tile([c.CHUNK, c.D], BF16, name="cc_y_out")
        x2_dram = dram.tile([c.CHUNK, c.D], F32, name="x2_dram")
        cc_aux_in = dram.tile([c.E, 2], F32, name="cc_aux_in")
        cc_aux_out = dram.tile([c.E, 2], F32, name="cc_aux_out",
                               addr_space="Shared")

        # ---------------- shared helpers ----------------
        def bcast_row(pl, name, width, tag="lnp", bufs=3, dt=F32):
            t = pl.tile([P, width], dt, name=f"br_{name}", tag=tag, bufs=bufs)
            nc.gpsimd.dma_start(out=t, in_=part_bcast(ins[name], P))
            return t

        def layernorm(src, dst, g_bc, b_bc, sqpool, stat):
            """token-major LN over free dim D (in-place safe: dst != src)."""
            sum_x = stat.tile([P, 1], F32, name="ln_sumx", tag="st")
            nc.vector.reduce_sum(sum_x, src, axis=AX.X)
            sqd = sqpool.tile([P, c.D], BF16, name="ln_sqd", tag="sq", bufs=2)
            sum_sq = stat.tile([P, 1], F32, name="ln_sumsq", tag="st")
            nc.vector.scalar_tensor_tensor(
                out=sqd, in0=src, scalar=1.0, in1=src,
                op0=ALU.mult, op1=ALU.mult, accum_out=sum_sq)
            mean = stat.tile([P, 1], F32, name="ln_mean", tag="st")
            nc.vector.tensor_scalar(out=mean, in0=sum_x, scalar1=1.0 / c.D,
                                    scalar2=None, op0=ALU.mult)
            msq = stat.tile([P, 1], F32, name="ln_msq", tag="st")
            nc.vector.tensor_tensor(out=msq, in0=mean, in1=mean, op=ALU.mult)
            # var + eps = sum_sq/D - mean^2 + eps
            var = stat.tile([P, 1], F32, name="ln_var", tag="st")
            nc.vector.tensor_scalar(out=var, in0=sum_sq, scalar1=1.0 / c.D,
                                    scalar2=1e-5, op0=ALU.mult, op1=ALU.add)
            vareps = stat.tile([P, 1], F32, name="ln_vareps", tag="st")
            nc.vector.tensor_tensor(out=vareps, in0=var, in1=msq,
                                    op=ALU.subtract)
            std = stat.tile([P, 1], F32, name="ln_std", tag="st")
            nc.scalar.sqrt(std, vareps)
            rstd = stat.tile([P, 1], F32, name="ln_rstd", tag="st")
            nc.vector.reciprocal(rstd, std)
            nc.vector.tensor_scalar(out=dst, in0=src, scalar1=mean,
                                    scalar2=rstd, op0=ALU.subtract,
                                    op1=ALU.mult)
            nc.vector.tensor_tensor(out=dst, in0=dst, in1=g_bc, op=ALU.mult)
            nc.vector.tensor_tensor(out=dst, in0=dst, in1=b_bc, op=ALU.add)

        def router_probs(lg_ap, scr, stat, out_ind=None, out_cmb=None):
            """Top-2 machinery from exact f32 logits. Gates are computed on
            unnormalized exp values (softmax denominator cancels in the
            renormalized top-2 gates); full softmax only when out_ind set."""
            m1a = stat.tile([P, 1], F32, name="rt_m1a", tag="st")
            nc.vector.reduce_max(m1a, lg_ap, axis=AX.X)
            negm = stat.tile([P, 1], F32, name="rt_negm", tag="st")
            nc.vector.tensor_scalar(out=negm, in0=m1a, scalar1=-1.0,
                                    scalar2=None, op0=ALU.mult)
            et = scr.tile([P, c.E], F32, name="rt_exp", tag="rtE", bufs=48)
            den = stat.tile([P, 1], F32, name="rt_den", tag="st")
            nc.scalar.activation(et, lg_ap, AF.Exp, bias=negm, accum_out=den)
            probs = None
            if out_ind is not None:
                rd = stat.tile([P, 1], F32, name="rt_rd", tag="st")
                nc.vector.reciprocal(rd, den)
                probs = scr.tile([P, c.E], F32, name="rt_probs", tag="rtE",
                                 bufs=48)
                nc.vector.tensor_scalar(out=probs, in0=et, scalar1=rd,
                                        scalar2=None, op0=ALU.mult)
                base = probs
            else:
                base = et
            m1 = stat.tile([P, 1], F32, name="rt_m1", tag="st")
            nc.vector.reduce_max(m1, base, axis=AX.X)
            lt = scr.tile([P, c.E], F32, name="rt_lt", tag="rtE", bufs=48)
            nc.vector.tensor_scalar(out=lt, in0=base, scalar1=m1,
                                    scalar2=None, op0=ALU.is_lt)
            pm = scr.tile([P, c.E], F32, name="rt_pm", tag="rtE", bufs=48)
            nc.vector.tensor_tensor(out=pm, in0=base, in1=lt, op=ALU.mult)
            m2 = stat.tile([P, 1], F32, name="rt_m2", tag="st")
            nc.vector.reduce_max(m2, pm, axis=AX.X)
            if out_ind is not None:
                nc.vector.tensor_scalar(out=out_ind, in0=base, scalar1=m2,
                                        scalar2=None, op0=ALU.is_ge)
            if out_cmb is not None:
                s12 = stat.tile([P, 1], F32, name="rt_s12", tag="st")
                nc.vector.tensor_tensor(out=s12, in0=m1, in1=m2, op=ALU.add)
                gsc = stat.tile([P, 1], F32, name="rt_gsc", tag="st")
                nc.vector.reciprocal(gsc, s12)
                pse = scr.tile([P, c.E], F32, name="rt_pse", tag="rtE",
                               bufs=48)
                nc.vector.tensor_tensor(out=pse, in0=base, in1=sel_bc,
                                        op=ALU.mult)
                pe = stat.tile([P, 1], F32, name="rt_pe", tag="st")
                nc.vector.reduce_sum(pe, pse, axis=AX.X)
                ind = stat.tile([P, 1], F32, name="rt_ind", tag="st")
                nc.vector.tensor_scalar(out=ind, in0=pe, scalar1=m2,
                                        scalar2=None, op0=ALU.is_ge)
                t1 = stat.tile([P, 1], F32, name="rt_t1", tag="st")
                nc.vector.tensor_tensor(out=t1, in0=pe, in1=gsc, op=ALU.mult)
                nc.vector.tensor_tensor(out=out_cmb, in0=t1, in1=ind,
                                        op=ALU.mult)
            return probs

        # ================= phases A + B: attention =================
        with tc.tile_pool(name="ab_big", bufs=1) as abp, \
             tc.tile_pool(name="ab_stat", bufs=96) as stat, \
             tc.tile_pool(name="ab_ps", bufs=5, space="PSUM") as ps_big, \
             tc.tile_pool(name="ab_psb", bufs=3, space="PSUM") as ps_small:

            def T(shape, dt, name, tag, bufs):
                return abp.tile(shape, dt, name=name, tag=tag, bufs=bufs)

            def bias_cols(name):
                t = abp.tile([P, c.D_T], F32, name=f"bc_{name}", tag="bcol",
                             bufs=8)
                nc.sync.dma_start(out=t,
                                  in_=ins[name].rearrange("(t p) -> p t", p=P))
                return t

            def attention(pfx, q_src, kv_dram, resid_in, ln_g, ln_b,
                          want_logits):
                tc.no_sync_barrier()
                EH = c.DH + 1
                bq, bk, bo = (bias_cols(pfx + "_bq"), bias_cols(pfx + "_bk"),
                              bias_cols(pfx + "_bo"))
                bv_bc = bcast_row(abp, pfx + "_bv", c.D)
                g_bc = bcast_row(abp, ln_g, c.D)
                b_bc = bcast_row(abp, ln_b, c.D)

                xt = []
                for k in range(c.D_T):
                    t = T([P, c.KV], BF16, f"{pfx}_xt{k}", "xt", c.D_T)
                    nc.sync.dma_start(out=t, in_=kv_dram[k * P:(k + 1) * P, :])
                    xt.append(t)

                def load_w_full(wname):
                    w = []
                    for k in range(c.D_T):
                        t = T([P, c.D], BF16, f"{wname}{k}", "wbig", c.D_T)
                        nc.sync.dma_start(out=t,
                                          in_=ins[wname][k * P:(k + 1) * P, :])
                        w.append(t)
                    return w

                # Q projection (fm)
                wq = load_w_full(pfx + "_wq")
                q_sb = []
                for m in range(c.D_T):
                    ps = ps_big.tile([P, c.CHUNK], F32, name="ps_q", tag="ps")
                    for k in range(c.D_T):
                        nc.tensor.matmul(ps, wq[k][:, m * P:(m + 1) * P],
                                         q_src[k], start=(k == 0),
                                         stop=(k == c.D_T - 1))
                    t = T([P, c.CHUNK], BF16, f"{pfx}_q{m}", "q", c.D_T)
                    nc.vector.tensor_scalar(out=t, in0=ps,
                                            scalar1=bq[:, m:m + 1],
                                            scalar2=None, op0=ALU.add)
                    q_sb.append(t)

                o_sb = [T([P, c.CHUNK], BF16, f"{pfx}_o{m}", "o", c.D_T)
                        for m in range(c.D_T)]

                for g in range(c.NG):
                    if g:
                        tc.no_sync_barrier()
                    gsl = slice(g * 512, (g + 1) * 512)
                    # K for this head group (fm rows g*512 .. +512)
                    wk_g = []
                    for k in range(c.D_T):
                        t = T([P, 512], BF16, f"{pfx}_wk{g}_{k}", "wsm",
                              c.D_T + 1)
                        nc.sync.dma_start(out=t,
                                          in_=ins[pfx + "_wk"][k * P:(k + 1) * P,
                                                               gsl])
                        wk_g.append(t)
                    k_g = []
                    for mi in range(512 // P):
                        t = T([P, c.KV], BF16, f"{pfx}_k{g}_{mi}", "k", 5)
                        for ns in range(c.KV_N):
                            nsl = slice(ns * 512, (ns + 1) * 512)
                            ps = ps_big.tile([P, 512], F32, name="ps_k",
                                             tag="ps")
                            for k in range(c.D_T):
                                nc.tensor.matmul(ps, wk_g[k][:, mi * P:(mi + 1) * P],
                                                 xt[k][:, nsl], start=(k == 0),
                                                 stop=(k == c.D_T - 1))
                            nc.vector.tensor_scalar(
                                out=t[:, nsl], in0=ps,
                                scalar1=bk[:, (g * 512 // P) + mi:
                                           (g * 512 // P) + mi + 1],
                                scalar2=None, op0=ALU.add)
                        k_g.append(t)
                    # V for this head group (tm, interleaved ones col)
                    wv_g = []
                    for k in range(c.D_T):
                        t = T([P, 512], BF16, f"{pfx}_wv{g}_{k}", "wsm",
                              c.D_T + 1)
                        nc.sync.dma_start(out=t,
                                          in_=ins[pfx + "_wv"][k * P:(k + 1) * P,
                                                               gsl])
                        wv_g.append(t)
                    v_g = []
                    for tt in range(c.KV_T):
                        t = T([P, c.HPG * EH], BF16, f"{pfx}_v{g}_{tt}", "v",
                              c.KV_T + 1)
                        v3 = t.rearrange("p (h e) -> p h e", e=EH)
                        nc.vector.memset(v3[:, :, c.DH:EH], 1.0)
                        ps = ps_big.tile([P, 512], F32, name="ps_v", tag="ps")
                        for k in range(c.D_T):
                            nc.tensor.matmul(ps, xt[k][:, tt * P:(tt + 1) * P],
                                             wv_g[k], start=(k == 0),
                                             stop=(k == c.D_T - 1))
                        view = v3[:, :, 0:c.DH]
                        nc.vector.tensor_copy(view, ps)
                        bvv = bv_bc[:, gsl].rearrange("p (h d) -> p h d",
                                                      d=c.DH)
                        nc.vector.tensor_tensor(out=view, in0=view, in1=bvv,
                                                op=ALU.add)
                        v_g.append(t)
                    # heads in this group
                    for hl in range(c.HPG):
                        h = g * c.HPG + hl
                        ki, ro = (hl * c.DH) // P, (hl * c.DH) % P
                        q_i, q_ro = (h * c.DH) // P, (h * c.DH) % P
                        q_ap = q_sb[q_i][q_ro:q_ro + c.DH, :]
                        ps_o = ps_big.tile([c.DH + 1, c.CHUNK], F32,
                                           name="ps_o", tag="ps")
                        av_insts = []
                        for kt in range(c.KV_T):
                            ps_s = ps_big.tile([P, c.CHUNK], F32, name="ps_s",
                                               tag="ps")
                            sc = nc.tensor.matmul(
                                ps_s, k_g[ki][ro:ro + c.DH, kt * P:(kt + 1) * P],
                                q_ap, start=True, stop=True)
                            if kt >= 3:
                                # keep scores <=3 ahead of AV so pt/ps slots
                                # can't wedge the pipeline
                                add_dep_helper(sc.ins, av_insts[kt - 3].ins,
                                               sync=False, reason="pt pacing")
                            ptt = T([P, c.CHUNK], BF16, "pt", "pt", 6)
                            nc.scalar.activation(ptt, ps_s, AF.Exp,
                                                 scale=float(1.0 / np.sqrt(c.DH)))
                            av = nc.tensor.matmul(
                                ps_o, v_g[kt][:, hl * EH:(hl + 1) * EH],
                                ptt, start=(kt == 0), stop=(kt == c.KV_T - 1))
                            av_insts.append(av)
                        recip = stat.tile([1, c.CHUNK], F32, name="att_recip",
                                          tag="arec", bufs=4)
                        nc.vector.reciprocal(recip, ps_o[c.DH:c.DH + 1, :])
                        ps_b = ps_small.tile([c.DH, c.CHUNK], F32, name="ps_b",
                                             tag="psb")
                        nc.tensor.matmul(ps_b, ones_row[0:1, 0:c.DH], recip,
                                         start=True, stop=True)
                        rbc = T([c.DH, c.CHUNK], BF16, "rbc", "abc", 4)
                        nc.scalar.copy(rbc, ps_b)
                        oi, oro = (h * c.DH) // P, (h * c.DH) % P
                        nc.vector.tensor_tensor(
                            out=o_sb[oi][oro:oro + c.DH, :],
                            in0=ps_o[0:c.DH, :], in1=rbc, op=ALU.mult)

                # out projection (fm, bf16) + transpose + residual + LN (tm)
                wo = load_w_full(pfx + "_wo")
                ao = []
                for m in range(c.D_T):
                    ps = ps_big.tile([P, c.CHUNK], F32, name="ps_ao", tag="ps")
                    for k in range(c.D_T):
                        nc.tensor.matmul(ps, wo[k][:, m * P:(m + 1) * P],
                                         o_sb[k], start=(k == 0),
                                         stop=(k == c.D_T - 1))
                    t = T([P, c.CHUNK], BF16, f"{pfx}_ao{m}", "ao", c.D_T)
                    nc.vector.tensor_scalar(out=t, in0=ps,
                                            scalar1=bo[:, m:m + 1],
                                            scalar2=None, op0=ALU.add)
                    ao.append(t)

                x_next = []
                for j in range(c.CH_T):
                    xpre = T([P, c.D], F32, f"{pfx}_xpre{j}", "resid", 5)
                    for m in range(c.D_T):
                        ps_t = ps_small.tile([P, P], BF16, name="ps_t",
                                             tag="psb")
                        nc.tensor.transpose(ps_t, ao[m][:, j * P:(j + 1) * P],
                                            ident_b)
                        nc.vector.tensor_tensor(
                            out=xpre[:, m * P:(m + 1) * P], in0=ps_t,
                            in1=resid_in[j][:, m * P:(m + 1) * P], op=ALU.add)
                    dst = T([P, c.D], F32, f"{pfx}_xn{j}", "resid", 5)
                    layernorm(xpre, dst, g_bc, b_bc, abp, stat)
                    if want_logits:
                        nc.sync.dma_start(
                            out=x2_dram[j * P:(j + 1) * P, :], in_=dst)
                    x_next.append(dst)

                # transpose x_next -> fm bf16 (+ f32 logits if requested)
                x_fm_bf = [T([P, c.CHUNK], BF16, f"{pfx}_xfmb{m}", "xfm",
                             c.D_T) for m in range(c.D_T)]
                for j in range(c.CH_T):
                    tmp32 = []
                    for m in range(c.D_T):
                        ps_t = ps_small.tile([P, P], F32, name="ps_t2",
                                             tag="psb")
                        nc.tensor.transpose(ps_t,
                                            x_next[j][:, m * P:(m + 1) * P],
                                            ident_f)
                        nc.scalar.copy(x_fm_bf[m][:, j * P:(j + 1) * P], ps_t)
                        if want_logits:
                            tf = T([P, P], F32, f"t32_{m}", "t32", c.D_T + 1)
                            nc.vector.tensor_copy(tf, ps_t)
                            tmp32.append(tf)
                    if want_logits:
                        ps_lg = ps_small.tile([P, c.E], F32, name="ps_lg",
                                              tag="psb")
                        for k in range(c.D_T):
                            nc.tensor.matmul(ps_lg, tmp32[k], rw_sb[k],
                                             start=(k == 0),
                                             stop=(k == c.D_T - 1))
                        lg = abp.tile([P, c.E], F32, name="lg_sb", tag="rtE",
                                      bufs=48)
                        nc.vector.tensor_tensor(out=lg, in0=ps_lg, in1=rb_bc,
                                                op=ALU.add)
                        nc.sync.dma_start(out=cc_lg_in[j * P:(j + 1) * P, :],
                                          in_=lg)
                return x_next, x_fm_bf

            # ---- run the two attention phases ----
            xq = []
            for k in range(c.D_T):
                t = abp.tile([P, c.CHUNK], BF16, name=f"xq{k}", tag="xq",
                             bufs=c.D_T)
                nc.sync.dma_start(out=t, in_=ins["xq"][k * P:(k + 1) * P, :])
                xq.append(t)
            x_tm = []
            for j in range(c.CH_T):
                t = abp.tile([P, c.D], F32, name=f"xtm{j}", tag="resid",
                             bufs=5)
                nc.sync.dma_start(out=t, in_=ins["x_tm"][j * P:(j + 1) * P, :])
                x_tm.append(t)

            x1_tm, x1_fm = attention("sa", xq, ins["xkv"], x_tm,
                                     "n1_g", "n1_b", False)
            x2_tm, x2_fm = attention("ca", x1_fm, ins["enc"], x1_tm,
                                     "n2_g", "n2_b", True)

            if c.routed_cap:
                for j in range(c.CH_T):
                    xtb = abp.tile([P, c.D], BF16, name=f"xtb{j}", tag="sq",
                                   bufs=2)
                    nc.vector.tensor_copy(xtb, x2_tm[j])
                    nc.sync.dma_start(out=cc_xtm_in[j * P:(j + 1) * P, :],
                                      in_=xtb)
            else:
                for m in range(c.D_T):
                    nc.sync.dma_start(out=cc_ag_in[m * P:(m + 1) * P, :],
                                      in_=x2_fm[m])

        if c.routed_cap:
            nc.gpsimd.collective_compute(
                "AllGather", ALU.bypass, replica_groups=[list(range(c.NC))],
                ins=[cc_xtm_in[:]], outs=[cc_xtm_out[:]])
        else:
            nc.gpsimd.collective_compute(
                "AllGather", ALU.bypass, replica_groups=[list(range(c.NC))],
                ins=[cc_ag_in[:]], outs=[cc_ag_out[:]])
        nc.gpsimd.collective_compute(
            "AllGather", ALU.bypass, replica_groups=[list(range(c.NC))],
            ins=[cc_lg_in[:]], outs=[cc_lg_out[:]])

        # ================= phase C: MoE (dense expert-parallel) =============
        with tc.tile_pool(name="moe", bufs=1) as mp, \
             tc.tile_pool(name="moe_stat", bufs=64) as mstat, \
             tc.tile_pool(name="moe_ps", bufs=4, space="PSUM") as mps, \
             tc.tile_pool(name="moe_psb", bufs=2, space="PSUM") as mpsb:

            w1_sb = []
            for k in range(c.D_T):
                t = mp.tile([P, c.F], BF16, name=f"w1sb{k}", tag="w1",
                            bufs=c.D_T)
                nc.sync.dma_start(out=t, in_=ins["w1"][k * P:(k + 1) * P, :])
                w1_sb.append(t)
            w2_sb = []
            for f in range(c.F_T):
                t = mp.tile([P, c.D], BF16, name=f"w2sb{f}", tag="w2",
                            bufs=c.F_T)
                nc.sync.dma_start(out=t, in_=ins["w2"][f * P:(f + 1) * P, :])
                w2_sb.append(t)
            b1_sb = mp.tile([P, c.F_T], F32, name="b1_sb", tag="b1", bufs=1)
            nc.sync.dma_start(out=b1_sb,
                              in_=ins["b1"].rearrange("(t p) -> p t", p=P))
            b2_bc = bcast_row(mp, "b2", c.D, tag="lnp", bufs=1, dt=BF16)

            def ffn_block(xsel, cmbg, scatter_idx):
                """relu(x@w1+b1) @ w2 + b2, gated, scattered to cc_y_in."""
                h_bf = []
                for f in range(c.F_T):
                    ps = mps.tile([P, 512], F32, name="ps_h", tag="ps")
                    for k in range(c.D_T):
                        nc.tensor.matmul(ps, w1_sb[k][:, f * P:(f + 1) * P],
                                         xsel[k], start=(k == 0),
                                         stop=(k == c.D_T - 1))
                    t = mp.tile([P, 512], BF16, name=f"h{f}", tag="h",
                                bufs=c.F_T)
                    nc.scalar.activation(t, ps, AF.Relu,
                                         bias=b1_sb[:, f:f + 1])
                    h_bf.append(t)
                for tt in range(4):
                    yrow = mp.tile([P, c.D], BF16, name="yrow", tag="y", bufs=2)
                    for dn in range(c.D_N):
                        ps = mps.tile([P, 512], F32, name="ps_y", tag="ps")
                        for f in range(c.F_T):
                            nc.tensor.matmul(
                                ps, h_bf[f][:, tt * P:(tt + 1) * P],
                                w2_sb[f][:, dn * 512:(dn + 1) * 512],
                                start=(f == 0), stop=(f == c.F_T - 1))
                        nc.vector.tensor_tensor(
                            out=yrow[:, dn * 512:(dn + 1) * 512], in0=ps,
                            in1=b2_bc[:, dn * 512:(dn + 1) * 512], op=ALU.add)
                    y2 = mp.tile([P, c.D], BF16, name="y2", tag="y2", bufs=2)
                    nc.vector.tensor_scalar(out=y2, in0=yrow,
                                            scalar1=cmbg[tt], scalar2=None,
                                            op0=ALU.mult)
                    nc.gpsimd.indirect_dma_start(
                        out=cc_y_in[:],
                        out_offset=bass.IndirectOffsetOnAxis(
                            ap=scatter_idx[tt][:, 0:1], axis=0),
                        in_=y2[:], in_offset=None,
                        bounds_check=c.NTOK - 1, oob_is_err=False)

            if c.routed_cap:
                CAP = c.routed_cap
                GRP = c.NTOK // P
                # zero the bf16 y buffer (RS sums all ranks)
                zt = mp.tile([P, 512], BF16, name="zt", tag="zt", bufs=1)
                nc.vector.memset(zt, 0.0)
                for r in range(GRP):
                    for dn in range(c.D_N):
                        nc.sync.dma_start(
                            out=cc_y_in[r * P:(r + 1) * P,
                                        dn * 512:(dn + 1) * 512], in_=zt)
                # slots table starts as all-dropped
                padt = mp.tile([P, 1], I32, name="padt", tag="small", bufs=48)
                nc.vector.memset(padt, 2 * c.NTOK)
                for s in range(CAP // P):
                    nc.sync.dma_start(out=slots_dram[s * P:(s + 1) * P, :],
                                      in_=padt)
                # routing for all tokens; flags[p, g] = token g*128+p selected
                flags = mp.tile([P, GRP], F32, name="flags", tag="flags",
                                bufs=1)
                for g in range(GRP):
                    row = g * P
                    lg = mp.tile([P, c.E], F32, name="moe_lg", tag="rtE",
                                 bufs=48)
                    nc.sync.dma_start(out=lg, in_=cc_lg_out[row:row + P, :])
                    cm = mp.tile([P, 1], F32, name="cmr", tag="cmb",
                                 bufs=40)
                    router_probs(lg, mp, mstat, out_cmb=cm)
                    nc.sync.dma_start(out=cmb_dram[row:row + P, :], in_=cm)
                    nc.vector.tensor_scalar(out=flags[:, g:g + 1], in0=cm,
                                            scalar1=0.0, scalar2=None,
                                            op0=ALU.is_gt)
                # group counts -> exclusive group offsets
                ps_cnt = mpsb.tile([GRP, 1], F32, name="ps_cnt", tag="psb")
                nc.tensor.matmul(ps_cnt, flags, ones_col, start=True,
                                 stop=True)
                cnt_sb = mp.tile([GRP, 1], F32, name="cnt_sb", tag="small",
                                 bufs=48)
                nc.vector.tensor_copy(cnt_sb, ps_cnt)
                ps_cr = mpsb.tile([1, GRP], F32, name="ps_cr", tag="psb")
                nc.tensor.matmul(ps_cr, cnt_sb, ident_f[0:GRP, 0:GRP],
                                 start=True, stop=True)
                cnt_row = mp.tile([1, GRP], F32, name="cnt_row", tag="small",
                                  bufs=48)
                nc.vector.tensor_copy(cnt_row, ps_cr)
                zrow = mp.tile([1, GRP], F32, name="zrow", tag="small",
                               bufs=48)
                nc.vector.memset(zrow, 0.0)
                incl = mp.tile([1, GRP], F32, name="incl", tag="small",
                               bufs=48)
                nc.vector.tensor_tensor_scan(out=incl, data0=cnt_row,
                                             data1=zrow, initial=0.0,
                                             op0=ALU.add, op1=ALU.add)
                goff = mp.tile([1, GRP], F32, name="goff", tag="small",
                               bufs=48)
                nc.vector.tensor_tensor(out=goff, in0=incl, in1=cnt_row,
                                        op=ALU.subtract)
                # per-token slot = within-group exclusive prefix + group offset
                ps_pos = mps.tile([P, GRP], F32, name="ps_pos", tag="ps")
                nc.tensor.matmul(ps_pos, utri, flags, start=True, stop=False)
                nc.tensor.matmul(ps_pos, ones_row[0:1, :], goff, start=False,
                                 stop=True)
                notf = mp.tile([P, GRP], F32, name="notf", tag="notf",
                               bufs=1)
                nc.vector.tensor_scalar(out=notf, in0=flags, scalar1=0.5,
                                        scalar2=float(3 * c.NTOK),
                                        op0=ALU.is_lt, op1=ALU.mult)
                posm = mp.tile([P, GRP], F32, name="posm", tag="posm",
                               bufs=1)
                nc.vector.tensor_tensor(out=posm, in0=ps_pos, in1=notf,
                                        op=ALU.add)
                for g in range(GRP):
                    pcol = mp.tile([P, 1], I32, name="pcol", tag="small",
                                   bufs=48)
                    nc.vector.tensor_copy(pcol, posm[:, g:g + 1])
                    tcol = mp.tile([P, 1], I32, name="tcol", tag="small",
                                   bufs=48)
                    nc.vector.tensor_scalar(out=tcol, in0=iota_col,
                                            scalar1=g * P, scalar2=None,
                                            op0=ALU.add)
                    nc.gpsimd.indirect_dma_start(
                        out=slots_dram[:],
                        out_offset=bass.IndirectOffsetOnAxis(
                            ap=pcol[:, 0:1], axis=0),
                        in_=tcol[:, 0:1], in_offset=None,
                        bounds_check=CAP - 1, oob_is_err=False)
                # gather + FFN + scatter per 512-token block of slots
                for sb in range(CAP // 512):
                    xsel = [mp.tile([P, 512], BF16, name=f"xb{k}", tag="xb",
                                    bufs=(3 * c.D_T) // 2) for k in range(c.D_T)]
                    cmbg, idxs = [], []
                    for g4 in range(4):
                        s0 = sb * 512 + g4 * P
                        idx = mp.tile([P, 1], I32, name="idx", tag="small",
                                      bufs=48)
                        nc.sync.dma_start(out=idx,
                                          in_=slots_dram[s0:s0 + P, :])
                        idxc = mp.tile([P, 1], I32, name="idxc", tag="small",
                                       bufs=48)
                        nc.vector.tensor_scalar(out=idxc, in0=idx,
                                                scalar1=c.NTOK - 1,
                                                scalar2=None, op0=ALU.min)
                        xg = mp.tile([P, c.D], BF16, name="xg", tag="xg",
                                     bufs=4)
                        nc.gpsimd.indirect_dma_start(
                            out=xg[:], out_offset=None, in_=cc_xtm_out[:],
                            in_offset=bass.IndirectOffsetOnAxis(
                                ap=idxc[:, 0:1], axis=0))
                        cg = mp.tile([P, 1], F32, name="cg", tag="cmb",
                                     bufs=40)
                        nc.gpsimd.indirect_dma_start(
                            out=cg[:], out_offset=None, in_=cmb_dram[:],
                            in_offset=bass.IndirectOffsetOnAxis(
                                ap=idxc[:, 0:1], axis=0))
                        cmbg.append(cg)
                        idxs.append(idx)
                        for k in range(c.D_T):
                            ps_t = mpsb.tile([P, P], BF16, name="ps_tx",
                                             tag="psb")
                            nc.tensor.transpose(ps_t, xg[:, k * P:(k + 1) * P],
                                                ident_b)
                            nc.scalar.copy(xsel[k][:, g4 * P:(g4 + 1) * P],
                                           ps_t)
                    ffn_block(xsel, cmbg, idxs)
            else:
                for blk in range(c.NC):
                    if blk:
                        tc.no_sync_barrier()
                    xb = []
                    for k in range(c.D_T):
                        t = mp.tile([P, c.CHUNK], BF16, name=f"xb{k}",
                                    tag="xb", bufs=c.D_T + 2)
                        nc.sync.dma_start(
                            out=t, in_=cc_ag_out[blk * c.D + k * P:
                                                 blk * c.D + (k + 1) * P, :])
                        xb.append(t)
                    cmb = []
                    for tt in range(c.CH_T):
                        lg = mp.tile([P, c.E], F32, name="moe_lg", tag="rtE",
                                     bufs=48)
                        row = blk * c.CHUNK + tt * P
                        nc.sync.dma_start(out=lg,
                                          in_=cc_lg_out[row:row + P, :])
                        cm = mp.tile([P, 1], F32, name=f"cmb{tt}", tag="cmb",
                                     bufs=40)
                        router_probs(lg, mp, mstat, out_cmb=cm)
                        cmb.append(cm)
                    h_bf = []
                    for f in range(c.F_T):
                        ps = mps.tile([P, c.CHUNK], F32, name="ps_h", tag="ps")
                        for k in range(c.D_T):
                            nc.tensor.matmul(ps,
                                             w1_sb[k][:, f * P:(f + 1) * P],
                                             xb[k], start=(k == 0),
                                             stop=(k == c.D_T - 1))
                        t = mp.tile([P, c.CHUNK], BF16, name=f"h{f}", tag="h",
                                    bufs=c.F_T + 2)
                        nc.scalar.activation(t, ps, AF.Relu,
                                             bias=b1_sb[:, f:f + 1])
                        h_bf.append(t)
                    for tt in range(c.CH_T):
                        for dn in range(c.D_N):
                            ps = mps.tile([P, 512], F32, name="ps_y", tag="ps")
                            for f in range(c.F_T):
                                nc.tensor.matmul(
                                    ps, h_bf[f][:, tt * P:(tt + 1) * P],
                                    w2_sb[f][:, dn * 512:(dn + 1) * 512],
                                    start=(f == 0), stop=(f == c.F_T - 1))
                            yt = mp.tile([P, 512], F32, name="yt", tag="y",
                                         bufs=4)
                            nc.vector.tensor_tensor(
                                out=yt, in0=ps,
                                in1=b2_bc[:, dn * 512:(dn + 1) * 512],
                                op=ALU.add)
                            yt2 = mp.tile([P, 512], BF16, name="yt2",
                                          tag="y2", bufs=4)
                            nc.vector.tensor_scalar(out=yt2, in0=yt,
                                                    scalar1=cmb[tt],
                                                    scalar2=None,
                                                    op0=ALU.mult)
                            row = blk * c.CHUNK + tt * P
                            nc.sync.dma_start(
                                out=cc_y_in[row:row + P,
                                            dn * 512:(dn + 1) * 512],
                                in_=yt2)

            nc.gpsimd.collective_compute(
                "ReduceScatter", ALU.add, replica_groups=[list(range(c.NC))],
                ins=[cc_y_in[:]], outs=[cc_y_out[:]])

        # ================= tail: LN3 + aux loss =================
        with tc.tile_pool(name="tl", bufs=1) as tp, \
             tc.tile_pool(name="tl_stat", bufs=96) as tstat, \
             tc.tile_pool(name="tl_ps", bufs=2, space="PSUM") as tps, \
             tc.tile_pool(name="tl_psb", bufs=2, space="PSUM") as tpsb:

            g3_bc = bcast_row(tp, "n3_g", c.D, tag="lnp", bufs=2)
            b3_bc = bcast_row(tp, "n3_b", c.D, tag="lnp", bufs=2)

            x3_tm = []
            for j in range(c.CH_T):
                yj = tp.tile([P, c.D], BF16, name=f"yj{j}", tag="yj", bufs=4)
                nc.sync.dma_start(out=yj, in_=cc_y_out[j * P:(j + 1) * P, :])
                x2j = tp.tile([P, c.D], F32, name=f"x2j{j}", tag="resid",
                              bufs=6)
                nc.sync.dma_start(out=x2j, in_=x2_dram[j * P:(j + 1) * P, :])
                x3pre = tp.tile([P, c.D], F32, name=f"x3pre{j}", tag="resid",
                                bufs=6)
                nc.vector.tensor_tensor(out=x3pre, in0=yj, in1=x2j,
                                        op=ALU.add)
                x3 = tp.tile([P, c.D], F32, name=f"x3_{j}", tag="x3",
                             bufs=c.CH_T)
                layernorm(x3pre, x3, g3_bc, b3_bc, tp, tstat)
                nc.sync.dma_start(out=outs["out_x"][j * P:(j + 1) * P, :],
                                  in_=x3)
                x3_tm.append(x3)

            ps_imp = tps.tile([c.E, 1], F32, name="ps_imp", tag="acc")
            ps_load = tps.tile([c.E, 1], F32, name="ps_load", tag="acc")
            for j in range(c.CH_T):
                tmp32 = []
                for m in range(c.D_T):
                    ps_t = tpsb.tile([P, P], F32, name="ps_t3", tag="psb")
                    nc.tensor.transpose(ps_t, x3_tm[j][:, m * P:(m + 1) * P],
                                        ident_f)
                    tf = tp.tile([P, P], F32, name=f"t33_{m}", tag="t32",
                                 bufs=c.D_T + 2)
                    nc.vector.tensor_copy(tf, ps_t)
                    tmp32.append(tf)
                ps_lg = tpsb.tile([P, c.E], F32, name="ps_lg3", tag="psb")
                for k in range(c.D_T):
                    nc.tensor.matmul(ps_lg, tmp32[k], rw_sb[k],
                                     start=(k == 0), stop=(k == c.D_T - 1))
                lg = tp.tile([P, c.E], F32, name="lg3", tag="rtE", bufs=48)
                nc.vector.tensor_tensor(out=lg, in0=ps_lg, in1=rb_bc,
                                        op=ALU.add)
                ind3 = tp.tile([P, c.E], F32, name="ind3", tag="rtE", bufs=48)
                probs3 = router_probs(lg, tp, tstat, out_ind=ind3)
                nc.tensor.matmul(ps_imp, probs3, ones_col, start=(j == 0),
                                 stop=(j == c.CH_T - 1))
                nc.tensor.matmul(ps_load, ind3, ones_col, start=(j == 0),
                                 stop=(j == c.CH_T - 1))

            aux_sb = tp.tile([c.E, 2], F32, name="aux_sb", tag="aux", bufs=4)
            nc.vector.tensor_copy(aux_sb[:, 0:1], ps_imp)
            nc.vector.tensor_copy(aux_sb[:, 1:2], ps_load)
            nc.sync.dma_start(out=cc_aux_in[:], in_=aux_sb)
            nc.gpsimd.collective_compute(
                "AllReduce", ALU.add, replica_groups=[list(range(c.NC))],
                ins=[cc_aux_in[:]], outs=[cc_aux_out[:]])
            ax = tp.tile([c.E, 2], F32, name="ax", tag="aux", bufs=4)
            nc.sync.dma_start(out=ax, in_=cc_aux_out[:])
            prod = tp.tile([c.E, 1], F32, name="prod", tag="aux", bufs=4)
            nc.vector.tensor_tensor(out=prod, in0=ax[:, 0:1], in1=ax[:, 1:2],
                                    op=ALU.mult)
            ps_f = tpsb.tile([1, 1], F32, name="ps_f", tag="psb")
            nc.tensor.matmul(ps_f, prod, ones_col[0:c.E, :], start=True,
                             stop=True)
            aux_fin = tp.tile([1, 1], F32, name="aux_fin", tag="aux", bufs=4)
            scale = float(c.E) / (float(c.NTOK) * float(c.NTOK * c.K))
            nc.scalar.activation(aux_fin, ps_f, AF.Copy, scale=scale)
            nc.sync.dma_start(out=outs["out_aux"][:], in_=aux_fin)


# ======================= host-side helpers =======================

def prepare_in_maps(inputs, cfg):
    """Full (unsharded) numpy inputs -> per-core in_maps."""
    c = cfg
    bf = ml_dtypes.bfloat16
    x = np.asarray(inputs["x"], np.float32)
    enc = np.asarray(inputs["encoder_output"], np.float32)
    per_batch = c.NC // c.B
    in_maps = []
    shared = {}
    for p in ("sa", "ca"):
        for m in ("wq", "wk", "wv", "wo"):
            shared[f"{p}_{m}"] = np.ascontiguousarray(
                np.asarray(inputs[f"{p}_{m}"], np.float32).astype(bf))
            shared[f"{p}_b{m[1]}"] = np.asarray(inputs[f"{p}_b{m[1]}"],
                                                np.float32)
    for n in ("n1", "n2", "n3"):
        shared[n + "_g"] = np.asarray(inputs[n + "_g"], np.float32)
        shared[n + "_b"] = np.asarray(inputs[n + "_b"], np.float32)
    shared["rw"] = np.asarray(inputs["r_w"], np.float32)
    shared["rb"] = np.asarray(inputs["r_b"], np.float32)
    e_w1 = np.asarray(inputs["e_w1"], np.float32)
    e_b1 = np.asarray(inputs["e_b1"], np.float32)
    e_w2 = np.asarray(inputs["e_w2"], np.float32)
    e_b2 = np.asarray(inputs["e_b2"], np.float32)
    for core in range(c.NC):
        b = core // per_batch
        q0 = (core % per_batch) * c.CHUNK
        m = dict(shared)
        m["xkv"] = np.ascontiguousarray(x[b].T.astype(bf))
        m["xq"] = np.ascontiguousarray(x[b, q0:q0 + c.CHUNK].T.astype(bf))
        m["x_tm"] = np.ascontiguousarray(x[b, q0:q0 + c.CHUNK])
        m["enc"] = np.ascontiguousarray(enc[b].T.astype(bf))
        m["w1"] = np.ascontiguousarray(e_w1[core].astype(bf))
        m["b1"] = np.ascontiguousarray(e_b1[core])
        m["w2"] = np.ascontiguousarray(e_w2[core].astype(bf))
        m["b2"] = np.ascontiguousarray(e_b2[core])
        sel = np.zeros((c.E,), np.float32)
        sel[core] = 1.0
        m["sel"] = sel
        in_maps.append(m)
    return in_maps


def input_specs(cfg):
    """name -> (shape, mybir dtype) for declaring dram tensors."""
    c = cfg
    sp = {
        "xkv": ([c.D, c.KV], BF16), "xq": ([c.D, c.CHUNK], BF16),
        "x_tm": ([c.CHUNK, c.D], F32), "enc": ([c.D, c.KV], BF16),
        "rw": ([c.D, c.E], F32), "rb": ([c.E], F32),
        "w1": ([c.D, c.F], BF16), "b1": ([c.F], F32),
        "w2": ([c.F, c.D], BF16), "b2": ([c.D], F32),
        "sel": ([c.E], F32),
    }
    for p in ("sa", "ca"):
        for m in ("wq", "wk", "wv", "wo"):
            sp[f"{p}_{m}"] = ([c.D, c.D], BF16)
            sp[f"{p}_b{m[1]}"] = ([c.D], F32)
    for n in ("n1", "n2", "n3"):
        sp[n + "_g"] = ([c.D], F32)
        sp[n + "_b"] = ([c.D], F32)
    return sp


def gather_outputs(results, cfg):
    c = cfg
    per_batch = c.NC // c.B
    x_full = np.zeros((c.B, c.S, c.D), np.float32)
    for core in range(c.NC):
        b = core // per_batch
        q0 = (core % per_batch) * c.CHUNK
        x_full[b, q0:q0 + c.CHUNK] = results[core]["out_x"]
    aux = np.float32(results[0]["out_aux"][0, 0])
    return x_full, aux




# ======================= public entry point =======================

_COMPILED = None
LAST_EXEC_NS = None


def _get_compiled():
    global _COMPILED
    if _COMPILED is None:
        from concourse import bacc
        cfg = Cfg(routed_cap=1536)
        nc = bacc.Bacc("TRN2", target_bir_lowering=False, debug=False,
                       num_devices=cfg.NC)
        ins = {name: nc.dram_tensor(name, shape, dt,
                                    kind="ExternalInput").ap()
               for name, (shape, dt) in input_specs(cfg).items()}
        outs = {
            "out_x": nc.dram_tensor("out_x", [cfg.CHUNK, cfg.D], F32,
                                    kind="ExternalOutput").ap(),
            "out_aux": nc.dram_tensor("out_aux", [1, 1], F32,
                                      kind="ExternalOutput").ap(),
        }
        with tile.TileContext(nc) as tc:
            build_kernel(tc, outs, ins, cfg)
        nc.compile()
        _COMPILED = (nc, cfg)
    return _COMPILED


def kernel(**inputs):
    from concourse.bass_utils import run_bass_kernel_spmd
    nc, cfg = _get_compiled()
    in_maps = prepare_in_maps(inputs, cfg)
    res = run_bass_kernel_spmd(nc, in_maps, core_ids=list(range(cfg.NC)))
    kernel.last_exec_time_ns = getattr(res, "exec_time_ns", None)
    global LAST_EXEC_NS
    LAST_EXEC_NS = kernel.last_exec_time_ns
    return gather_outputs(res.results, cfg)
